# revision 1
# baseline (speedup 1.0000x reference)
"""Multi-head attention (B=2, S=2048, D=1024, H=16, Dk=64) on 8 NeuronCores.

Sharding: 2-way data parallel over batch x 4-way tensor parallel over heads.
Core c handles batch c//4 and heads (c%4)*4 .. (c%4)*4+3, i.e. a 256-column
slice of the QKV projections and the matching 256-row slice of Wo. Each core
computes a partial output projection [S, D]; the host sums the 4 partials per
batch (the all-reduce of the sharding hint) and stacks the batches.

On-core algorithm (matmuls in float32r = full-rate fp32, PSUM accum fp32):
  x^T via PE transpose -> Q^T, K^T head-packed [128, 2, S] (head parity on
  partition halves 0-63/64-127 so the two heads' K=64 score matmuls run
  concurrently in separate PE row groups) and V in natural [t, d'] layout,
  augmented with a ones column -> S^T = K_h Q_h^T -> exp on ACT (1/8 scale
  folded into the activation; no max subtraction needed: scores are O(5)
  for unit-variance inputs, far from fp32 overflow) -> C^T = V_aug^T @
  expS^T where the ones row yields the softmax denominator for free ->
  normalize -> partial out = C^T.T @ Wo_slice + bo/4.

Emission order interleaves the V projection and the j+1 Q projection into
attention block j so the PE keeps dense work while ACT grinds through the
exp stream; x/output DMAs are split across the SP/ACT HWDGE queues and
weight loads go via the gpsimd SWDGE queues.
"""
from contextlib import ExitStack

import numpy as np
import concourse.bass as bass
import concourse.mybir as mybir
import concourse.tile as tile
from concourse import bacc
from concourse.bass_utils import run_bass_kernel_spmd
from concourse.masks import make_identity

f32 = mybir.dt.float32
f32r = mybir.dt.float32r
AF = mybir.ActivationFunctionType
ALU = mybir.AluOpType

B, S, D = 2, 2048, 1024
H, DK = 16, 64
NCORES = 8
TP = 4                 # tensor-parallel factor (head groups)
HPC = H // TP          # 4 heads per core
DP = HPC * DK          # 256 = per-core d' slice
SBK = 512              # s-block for attention streaming
NSB = S // SBK         # 4
NT = S // 128          # 16 t-tiles
NDC = D // 128         # 8 contraction chunks over D
NPC = DP // 128        # 2 chunks over d'

_prog_cache = {}


def _build_program():
    nc = bacc.Bacc()
    x = nc.dram_tensor("x", [S, D], f32, kind="ExternalInput")
    wq = nc.dram_tensor("wq", [D, DP], f32, kind="ExternalInput")
    wk = nc.dram_tensor("wk", [D, DP], f32, kind="ExternalInput")
    wv = nc.dram_tensor("wv", [D, DP], f32, kind="ExternalInput")
    wo = nc.dram_tensor("wo", [DP, D], f32, kind="ExternalInput")
    bq = nc.dram_tensor("bq", [DP], f32, kind="ExternalInput")
    bk = nc.dram_tensor("bk", [DP], f32, kind="ExternalInput")
    bv = nc.dram_tensor("bv", [DP], f32, kind="ExternalInput")
    out = nc.dram_tensor("out", [S, D], f32, kind="ExternalOutput")

    with tile.TileContext(nc) as tc, ExitStack() as top:
        const = top.enter_context(tc.tile_pool(name="const", bufs=1))
        big = top.enter_context(tc.tile_pool(name="big", bufs=1))
        xtp = top.enter_context(tc.tile_pool(name="xt", bufs=1))

        ident = const.tile([128, 128], f32)
        make_identity(nc, ident)

        # persistent activations
        qt_r = big.tile([128, NPC, S], f32r)
        kt_r = big.tile([128, NPC, S], f32r)
        vaug = big.tile([128, NT, HPC, DK + 1], f32r)
        ct_r = big.tile([128, NPC, S], f32r)
        xt_r = xtp.tile([128, NDC, S], f32r)

        wq_r = const.tile([128, NDC, DP], f32r)
        wv_r = const.tile([128, NDC, DP], f32r)
        wo_r = const.tile([128, NPC, D], f32r)
        bq_sb = const.tile([128, NPC], f32)
        bk_sb = const.tile([128, NPC], f32)
        bv_b = const.tile([128, DP], f32)
        ones_f = const.tile([128, NT, HPC], f32)

        wkp = top.enter_context(tc.tile_pool(name="wkp", bufs=1))
        wk_r = wkp.tile([128, NDC, DP], f32r)

        es_ld = ExitStack()
        stg = es_ld.enter_context(tc.tile_pool(name="stg", bufs=2))
        xin = es_ld.enter_context(tc.tile_pool(name="xin", bufs=2))
        ps_t = es_ld.enter_context(tc.tile_pool(name="ps_t", bufs=2, space="PSUM"))

        # ---- loads: x tiles on the two HWDGE queues, weights on SWDGE ----
        x_tiles = []
        for st in range(NT):
            x_t = xin.tile([128, D], f32, tag="x_t", name=f"x_t{st}")
            eng = nc.sync if st % 2 == 0 else nc.scalar
            eng.dma_start(out=x_t, in_=x[st * 128:(st + 1) * 128, :])
            x_tiles.append(x_t)

        wstg = {}
        for src, npc, nm in ((wq, NDC, "wq"), (wk, NDC, "wk"),
                             (wv, NDC, "wv"), (wo, NPC, "wo")):
            sf = stg.tile([128, npc, src.shape[1]], f32, tag="wstg", name=f"stg_{nm}")
            nc.gpsimd.dma_start(out=sf, in_=src.rearrange("(ko ki) d -> ki ko d", ki=128))
            wstg[nm] = sf
        nc.gpsimd.dma_start(out=bq_sb, in_=bq[:].rearrange("(c p) -> p c", p=128))
        nc.gpsimd.dma_start(out=bk_sb, in_=bk[:].rearrange("(c p) -> p c", p=128))
        bv_1 = const.tile([1, DP], f32)
        nc.gpsimd.dma_start(out=bv_1, in_=bv[:].rearrange("(a d) -> a d", a=1))
        nc.gpsimd.partition_broadcast(bv_b, bv_1)

        nc.vector.tensor_copy(out=wq_r, in_=wstg["wq"])
        nc.vector.tensor_copy(out=wk_r, in_=wstg["wk"])

        def proj_qk(pool, wr, bias_sb, dst, c, j):
            pq = pool.tile([128, SBK], f32, tag="pqkv", name=f"pj{c}_{j}_{id(wr)%89}")
            for k in range(NDC):
                nc.tensor.matmul(
                    out=pq,
                    lhsT=wr[:, k, c * 128:(c + 1) * 128],
                    rhs=xt_r[:, k, j * SBK:(j + 1) * SBK],
                    start=(k == 0), stop=(k == NDC - 1),
                )
            nc.vector.tensor_scalar_add(
                out=dst[:, c, j * SBK:(j + 1) * SBK],
                in0=pq, scalar1=bias_sb[:, c:c + 1],
            )

        def proj_v(pool, st):
            pv = pool.tile([128, DP], f32, tag="px", name=f"pv{st}")
            for k in range(NDC):
                nc.tensor.matmul(
                    out=pv,
                    lhsT=xt_r[:, k, st * 128:(st + 1) * 128],
                    rhs=wv_r[:, k, :],
                    start=(k == 0), stop=(k == NDC - 1),
                )
            nc.vector.tensor_add(
                out=vaug[:, st, :, 0:DK],
                in0=pv.rearrange("p (h d) -> p h d", h=HPC),
                in1=bv_b.rearrange("p (h d) -> p h d", h=HPC),
            )

        # ---- x^T via PE transpose, interleaved with the c=0 K projections
        # so the first attention block can start as early as possible ----
        ps_p = es_ld.enter_context(tc.tile_pool(name="ps_p", bufs=2, space="PSUM"))
        for g in range(4):
            for st in range(4 * g, 4 * g + 4):
                tp = ps_t.tile([128, NDC * 128], f32, tag="tp", name=f"tp{st}")
                for k in range(NDC):
                    nc.tensor.transpose(
                        out=tp[:, k * 128:(k + 1) * 128],
                        in_=x_tiles[st][:, k * 128:(k + 1) * 128],
                        identity=ident,
                    )
                nc.vector.tensor_copy(
                    out=xt_r[:, :, st * 128:(st + 1) * 128],
                    in_=tp.rearrange("p (k s) -> p k s", k=NDC),
                )
            proj_qk(ps_p, wk_r, bk_sb, kt_r, 0, g)
        proj_qk(ps_p, wq_r, bq_sb, qt_r, 0, 0)

        nc.vector.tensor_copy(out=wv_r, in_=wstg["wv"])
        nc.vector.tensor_copy(out=wo_r, in_=wstg["wo"])
        nc.vector.memset(ones_f, 1.0)
        nc.vector.tensor_copy(out=vaug[:, :, :, DK], in_=ones_f)

        es_ld.close()   # frees stg + xin SBUF and the phase-1 PSUM banks

        # ---- attention + output projection ----
        with ExitStack() as ph2:
            esp = ph2.enter_context(tc.tile_pool(name="esp", bufs=3))
            smal = ph2.enter_context(tc.tile_pool(name="smal", bufs=2))
            outp = ph2.enter_context(tc.tile_pool(name="outp", bufs=2))
            ps_x = ph2.enter_context(tc.tile_pool(name="ps_x", bufs=2, space="PSUM"))
            ps_s = ph2.enter_context(tc.tile_pool(name="ps_s", bufs=2, space="PSUM"))
            ps_c = ph2.enter_context(tc.tile_pool(name="ps_c", bufs=1, space="PSUM"))

            # Filler queue: exp-independent PE work (out-projections, next-j Q
            # projections) dripped into the attention t-loop so the PE never
            # idles waiting on ACT and the HAM clock stays warm.
            filler = []

            def feed(n):
                for _ in range(min(n, len(filler))):
                    filler.pop(0)()

            def flush():
                while filler:
                    filler.pop(0)()

            def queue_proj_qk(wr, bias_sb, dst, c, j):
                state = {}
                def mk(k):
                    def go():
                        if k == 0:
                            state["pq"] = ps_x.tile(
                                [128, SBK], f32, tag="px", name=f"fq{c}_{j}_{k}")
                        nc.tensor.matmul(
                            out=state["pq"],
                            lhsT=wr[:, k, c * 128:(c + 1) * 128],
                            rhs=xt_r[:, k, j * SBK:(j + 1) * SBK],
                            start=(k == 0), stop=(k == NDC - 1),
                        )
                        if k == NDC - 1:
                            nc.vector.tensor_scalar_add(
                                out=dst[:, c, j * SBK:(j + 1) * SBK],
                                in0=state["pq"], scalar1=bias_sb[:, c:c + 1],
                            )
                    return go
                for k in range(NDC):
                    filler.append(mk(k))

            def queue_outproj(j):
                for stj in range(SBK // 128):
                    st = j * (SBK // 128) + stj
                    for nh in range(2):
                        state = {}
                        def mk(c, st=st, nh=nh, state=state):
                            def go():
                                if c == 0:
                                    state["po"] = ps_x.tile(
                                        [128, 512], f32, tag="px", name=f"po{st}_{nh}")
                                nc.tensor.matmul(
                                    out=state["po"],
                                    lhsT=ct_r[:, c, st * 128:(st + 1) * 128],
                                    rhs=wo_r[:, c, nh * 512:(nh + 1) * 512],
                                    start=(c == 0), stop=(c == NPC - 1),
                                )
                                if c == NPC - 1:
                                    ob = outp.tile([128, 512], f32, tag="ob",
                                                   name=f"ob{st}_{nh}")
                                    nc.vector.tensor_copy(out=ob, in_=state["po"])
                                    eng = nc.sync if st % 2 == 0 else nc.scalar
                                    eng.dma_start(
                                        out=out[st * 128:(st + 1) * 128,
                                                nh * 512:(nh + 1) * 512],
                                        in_=ob)
                            return go
                        for c in range(NPC):
                            filler.append(mk(c))

            def normalize(j, hp, pcs):
                cus = []
                for hh in range(2):
                    cu = smal.tile([DK + 1, SBK], f32, tag=f"cu{hh}", bufs=1,
                                   name=f"cu{j}{hp}{hh}")
                    nc.vector.tensor_copy(out=cu, in_=pcs[hh])
                    cus.append(cu)
                # reciprocal in partition-major layout: [1,512] row -> [128,4]
                dnT = smal.tile([128, 4, 2], f32, tag="dnT", name=f"dnT{j}{hp}")
                for hh in range(2):
                    nc.gpsimd.dma_start(out=dnT[:, :, hh], in_=cus[hh][DK:DK + 1, :])
                rT = smal.tile([128, 4, 2], f32, tag="rT", name=f"rT{j}{hp}")
                nc.vector.reciprocal(out=rT, in_=dnT)
                for hh in range(2):
                    rr = smal.tile([1, SBK], f32, tag="rr", bufs=1, name=f"rr{j}{hp}{hh}")
                    nc.gpsimd.dma_start(out=rr, in_=rT[:, :, hh])
                    rb = smal.tile([64, SBK], f32, tag="rb", bufs=1, name=f"rb{j}{hp}{hh}")
                    nc.gpsimd.partition_broadcast(rb, rr)
                    nc.vector.tensor_mul(
                        out=ct_r[hh * 64:(hh + 1) * 64, hp, j * SBK:(j + 1) * SBK],
                        in0=cus[hh][0:DK, :],
                        in1=rb,
                    )

            # Software-pipelined driver (depth 2): at step i emit S(i), exp(i),
            # then the PV of step i-2 — whose exp finished two steps ago, so
            # the PE never sits in an exp-wait ahead of the next S and the exp
            # stream stays back-to-back.
            pend = []

            def drain_pv():
                if not pend:
                    return
                j, hp, t, es, pcs = pend.pop(0)
                for hh in range(2):
                    nc.tensor.matmul(
                        out=pcs[hh],
                        lhsT=vaug[:, t, hp * 2 + hh, :],
                        rhs=es[:, hh, :],
                        start=(t == 0), stop=(t == NT - 1),
                    )
                if t == NT - 1:
                    normalize(j, hp, pcs)
                    if hp == 1:
                        queue_outproj(j)

            pcs_by = {}
            for j in range(NSB):
                for hp in range(NPC):
                    pcs_by[(j, hp)] = [
                        ps_c.tile([DK + 1, SBK], f32, tag=f"pc{hh}", name=f"pc{hh}_{j}_{hp}")
                        for hh in range(2)]
                    if (j, hp) == (0, 0):
                        for jj in range(NSB):
                            queue_proj_qk(wk_r, bk_sb, kt_r, 1, jj)
                        queue_proj_qk(wq_r, bq_sb, qt_r, 1, 0)
                    if hp == 1 and j + 1 < NSB:
                        for c in range(NPC):
                            queue_proj_qk(wq_r, bq_sb, qt_r, c, j + 1)
                    for t in range(NT):
                        if j == 0 and hp == 0:
                            proj_v(ps_x, t)        # V projection rides along
                        ss = ps_s.tile([128, 2, SBK], f32, tag="ss", name=f"ss{j}_{hp}_{t}")
                        for hh in range(2):
                            nc.tensor.matmul(
                                out=ss[:, hh, :],
                                lhsT=kt_r[hh * 64:(hh + 1) * 64, hp, t * 128:(t + 1) * 128],
                                rhs=qt_r[hh * 64:(hh + 1) * 64, hp, j * SBK:(j + 1) * SBK],
                                start=True, stop=True,
                            )
                        es = esp.tile([128, 2, SBK], f32r, tag="es", name=f"es{j}_{hp}_{t}")
                        nc.scalar.activation(out=es, in_=ss, func=AF.Exp, scale=0.125)
                        if len(pend) >= 2:
                            drain_pv()
                        pend.append((j, hp, t, es, pcs_by[(j, hp)]))
                        feed(3)
            drain_pv()
            drain_pv()
            flush()

    nc.finalize()
    return nc


def _get_program():
    if "nc" not in _prog_cache:
        _prog_cache["nc"] = _build_program()
    return _prog_cache["nc"]


def _make_in_maps(x, Wq, bq, Wk, bk, Wv, bv, Wo, bo):
    in_maps = []
    for c in range(NCORES):
        b, hg = divmod(c, TP)
        sl = slice(hg * DP, (hg + 1) * DP)
        in_maps.append({
            "x": np.ascontiguousarray(x[b]),
            "wq": np.ascontiguousarray(Wq[:, sl]),
            "wk": np.ascontiguousarray(Wk[:, sl]),
            "wv": np.ascontiguousarray(Wv[:, sl]),
            "wo": np.ascontiguousarray(Wo[sl, :]),
            "bq": np.ascontiguousarray(bq[sl]),
            "bk": np.ascontiguousarray(bk[sl]),
            "bv": np.ascontiguousarray(bv[sl]),
        })
    return in_maps


def run(inputs, **spmd_kwargs):
    """Build, run on 8 cores, gather. Returns (output, BassKernelResults)."""
    args = {k: np.asarray(v, dtype=np.float32) for k, v in inputs.items()}
    nc = _get_program()
    in_maps = _make_in_maps(
        args["x"], args["Wq"], args["bq"], args["Wk"], args["bk"],
        args["Wv"], args["bv"], args["Wo"], args["bo"],
    )
    res = run_bass_kernel_spmd(nc, in_maps, list(range(NCORES)), **spmd_kwargs)
    out = np.zeros((B, S, D), dtype=np.float32)
    for c in range(NCORES):
        b = c // TP
        out[b] += res.results[c]["out"]
    out += args["bo"]
    return out, res


def kernel(**inputs):
    out, _ = run(inputs)
    return out



# revision 8
# speedup vs baseline: 1.0922x; 1.0922x over previous
"""Multi-head attention (B=2, S=2048, D=1024, H=16, Dk=64) on 8 NeuronCores.

Sharding: 2-way data parallel over batch x 4-way tensor parallel over heads.
Core c handles batch c//4 and heads (c%4)*4 .. (c%4)*4+3, i.e. a 256-column
slice of the QKV projections and the matching 256-row slice of Wo. Each core
computes a partial output projection [S, D]; the host sums the 4 partials per
batch (the all-reduce of the sharding hint) and stacks the batches.

On-core algorithm (matmuls in float32r = full-rate fp32, PSUM accum fp32):
  x^T via PE transpose -> Q^T, K^T head-packed [128, 2, S] (head parity on
  partition halves 0-63/64-127 so the two heads' K=64 score matmuls run
  concurrently in separate PE row groups) and V in natural [t, d'] layout,
  augmented with a ones column -> S^T = K_h Q_h^T -> exp on ACT (1/8 scale
  folded into the activation; no max subtraction needed: scores are O(5)
  for unit-variance inputs, far from fp32 overflow) -> C^T = V_aug^T @
  expS^T where the ones row yields the softmax denominator for free ->
  normalize -> partial out = C^T.T @ Wo_slice + bo/4.

Emission order interleaves the V projection and the j+1 Q projection into
attention block j so the PE keeps dense work while ACT grinds through the
exp stream; x/output DMAs are split across the SP/ACT HWDGE queues and
weight loads go via the gpsimd SWDGE queues.
"""
from contextlib import ExitStack

import numpy as np
import concourse.bass as bass
import concourse.mybir as mybir
import concourse.tile as tile
from concourse import bacc
from concourse.bass_utils import run_bass_kernel_spmd
from concourse.masks import make_identity

f32 = mybir.dt.float32
f32r = mybir.dt.float32r
bf16 = mybir.dt.bfloat16
AF = mybir.ActivationFunctionType
ALU = mybir.AluOpType

B, S, D = 2, 2048, 1024
H, DK = 16, 64
NCORES = 8
TP = 4                 # tensor-parallel factor (head groups)
HPC = H // TP          # 4 heads per core
DP = HPC * DK          # 256 = per-core d' slice
SBK = 512              # s-block for attention streaming
NSB = S // SBK         # 4
NT = S // 128          # 16 t-tiles
NDC = D // 128         # 8 contraction chunks over D
NPC = DP // 128        # 2 chunks over d'

_prog_cache = {}


def _build_program():
    nc = bacc.Bacc()
    x = nc.dram_tensor("x", [S, D], f32, kind="ExternalInput")
    wq = nc.dram_tensor("wq", [D, DP], f32, kind="ExternalInput")
    wk = nc.dram_tensor("wk", [D, DP], f32, kind="ExternalInput")
    wv = nc.dram_tensor("wv", [D, DP], f32, kind="ExternalInput")
    wo = nc.dram_tensor("wo", [DP, D], f32, kind="ExternalInput")
    bq = nc.dram_tensor("bq", [DP], f32, kind="ExternalInput")
    bk = nc.dram_tensor("bk", [DP], f32, kind="ExternalInput")
    bv = nc.dram_tensor("bv", [DP], f32, kind="ExternalInput")
    out = nc.dram_tensor("out", [S, D], f32, kind="ExternalOutput")

    with tile.TileContext(nc) as tc, ExitStack() as top:
        const = top.enter_context(tc.tile_pool(name="const", bufs=1))
        big = top.enter_context(tc.tile_pool(name="big", bufs=1))
        xtp = top.enter_context(tc.tile_pool(name="xt", bufs=1))

        ident = const.tile([128, 128], f32)
        make_identity(nc, ident)

        # persistent activations
        qt_r = big.tile([128, NPC, S], f32r)
        kt_r = big.tile([128, NPC, S], f32r)
        # V augmented with a 64-wide ones block: the PV matmul then emits the
        # softmax denominator replicated across PSUM partitions 64-127, so
        # normalization is a straight DVE reciprocal+mul from PSUM (no gpsimd
        # transpose/broadcast round-trips).
        vaug = big.tile([128, NT, HPC, 2 * DK], bf16)
        ct_r = big.tile([128, NPC, S], f32r)
        xt_r = xtp.tile([128, NDC, S], f32r)

        wq_r = const.tile([128, NDC, DP], f32r)
        wv_r = const.tile([128, NDC, DP], f32r)
        wo_r = const.tile([128, NPC, D], f32r)
        bq_sb = const.tile([128, NPC], f32)
        bk_sb = const.tile([128, NPC], f32)
        bv_b = const.tile([128, DP], f32)

        wkp = top.enter_context(tc.tile_pool(name="wkp", bufs=1))
        wk_r = wkp.tile([128, NDC, DP], f32r)

        es_ld = ExitStack()
        stg = es_ld.enter_context(tc.tile_pool(name="stg", bufs=2))
        xin = es_ld.enter_context(tc.tile_pool(name="xin", bufs=2))
        ps_t = es_ld.enter_context(tc.tile_pool(name="ps_t", bufs=2, space="PSUM"))

        # ---- loads: x tiles on the two HWDGE queues, weights on SWDGE ----
        x_tiles = []
        for st in range(NT):
            x_t = xin.tile([128, D], f32, tag="x_t", name=f"x_t{st}")
            eng = nc.sync if st % 2 == 0 else nc.scalar
            eng.dma_start(out=x_t, in_=x[st * 128:(st + 1) * 128, :])
            x_tiles.append(x_t)

        wstg = {}
        for src, npc, nm in ((wq, NDC, "wq"), (wk, NDC, "wk"),
                             (wv, NDC, "wv"), (wo, NPC, "wo")):
            sf = stg.tile([128, npc, src.shape[1]], f32, tag="wstg", name=f"stg_{nm}")
            nc.gpsimd.dma_start(out=sf, in_=src.rearrange("(ko ki) d -> ki ko d", ki=128))
            wstg[nm] = sf
        nc.gpsimd.dma_start(out=bq_sb, in_=bq[:].rearrange("(c p) -> p c", p=128))
        nc.gpsimd.dma_start(out=bk_sb, in_=bk[:].rearrange("(c p) -> p c", p=128))
        bv_1 = const.tile([1, DP], f32)
        nc.gpsimd.dma_start(out=bv_1, in_=bv[:].rearrange("(a d) -> a d", a=1))
        nc.gpsimd.partition_broadcast(bv_b, bv_1)

        nc.vector.tensor_copy(out=wq_r, in_=wstg["wq"])
        nc.vector.tensor_copy(out=wk_r, in_=wstg["wk"])

        def proj_qk(pool, wr, bias_sb, dst, c, j):
            pq = pool.tile([128, SBK], f32, tag="pqkv", name=f"pj{c}_{j}_{id(wr)%89}")
            for k in range(NDC):
                nc.tensor.matmul(
                    out=pq,
                    lhsT=wr[:, k, c * 128:(c + 1) * 128],
                    rhs=xt_r[:, k, j * SBK:(j + 1) * SBK],
                    start=(k == 0), stop=(k == NDC - 1),
                )
            nc.vector.tensor_scalar_add(
                out=dst[:, c, j * SBK:(j + 1) * SBK],
                in0=pq, scalar1=bias_sb[:, c:c + 1],
            )

        def proj_v(pool, st):
            pv = pool.tile([128, DP], f32, tag="px", name=f"pv{st}")
            for k in range(NDC):
                nc.tensor.matmul(
                    out=pv,
                    lhsT=xt_r[:, k, st * 128:(st + 1) * 128],
                    rhs=wv_r[:, k, :],
                    start=(k == 0), stop=(k == NDC - 1),
                )
            nc.vector.tensor_add(
                out=vaug[:, st, :, 0:DK],
                in0=pv.rearrange("p (h d) -> p h d", h=HPC),
                in1=bv_b.rearrange("p (h d) -> p h d", h=HPC),
            )

        # ---- x^T via PE transpose, interleaved with the c=0 K projections
        # so the first attention block can start as early as possible ----
        ps_p = es_ld.enter_context(tc.tile_pool(name="ps_p", bufs=2, space="PSUM"))
        for g in range(4):
            for st in range(4 * g, 4 * g + 4):
                tp = ps_t.tile([128, NDC * 128], f32, tag="tp", name=f"tp{st}")
                for k in range(NDC):
                    nc.tensor.transpose(
                        out=tp[:, k * 128:(k + 1) * 128],
                        in_=x_tiles[st][:, k * 128:(k + 1) * 128],
                        identity=ident,
                    )
                nc.vector.tensor_copy(
                    out=xt_r[:, :, st * 128:(st + 1) * 128],
                    in_=tp.rearrange("p (k s) -> p k s", k=NDC),
                )
            proj_qk(ps_p, wk_r, bk_sb, kt_r, 0, g)
        proj_qk(ps_p, wq_r, bq_sb, qt_r, 0, 0)

        nc.vector.tensor_copy(out=wv_r, in_=wstg["wv"])
        nc.vector.tensor_copy(out=wo_r, in_=wstg["wo"])
        nc.vector.memset(vaug[:, :, :, DK:2 * DK], 1.0)

        es_ld.close()   # frees stg + xin SBUF and the phase-1 PSUM banks

        # ---- attention + output projection ----
        with ExitStack() as ph2:
            esp = ph2.enter_context(tc.tile_pool(name="esp", bufs=3))
            smal = ph2.enter_context(tc.tile_pool(name="smal", bufs=2))
            outp = ph2.enter_context(tc.tile_pool(name="outp", bufs=2))
            ps_x = ph2.enter_context(tc.tile_pool(name="ps_x", bufs=2, space="PSUM"))
            ps_s = ph2.enter_context(tc.tile_pool(name="ps_s", bufs=2, space="PSUM"))
            ps_c = ph2.enter_context(tc.tile_pool(name="ps_c", bufs=1, space="PSUM"))

            # Filler queue: exp-independent PE work (out-projections, next-j Q
            # projections) dripped into the attention t-loop so the PE never
            # idles waiting on ACT and the HAM clock stays warm.
            filler = []

            def feed(n):
                for _ in range(min(n, len(filler))):
                    filler.pop(0)()

            def flush():
                while filler:
                    filler.pop(0)()

            def queue_proj_qk(wr, bias_sb, dst, c, j):
                state = {}
                def mk(k):
                    def go():
                        if k == 0:
                            state["pq"] = ps_x.tile(
                                [128, SBK], f32, tag="px", name=f"fq{c}_{j}_{k}")
                        nc.tensor.matmul(
                            out=state["pq"],
                            lhsT=wr[:, k, c * 128:(c + 1) * 128],
                            rhs=xt_r[:, k, j * SBK:(j + 1) * SBK],
                            start=(k == 0), stop=(k == NDC - 1),
                        )
                        if k == NDC - 1:
                            nc.vector.tensor_scalar_add(
                                out=dst[:, c, j * SBK:(j + 1) * SBK],
                                in0=state["pq"], scalar1=bias_sb[:, c:c + 1],
                            )
                    return go
                for k in range(NDC):
                    filler.append(mk(k))

            def queue_outproj(j):
                for stj in range(SBK // 128):
                    st = j * (SBK // 128) + stj
                    for nh in range(2):
                        state = {}
                        def mk(c, st=st, nh=nh, state=state):
                            def go():
                                if c == 0:
                                    state["po"] = ps_x.tile(
                                        [128, 512], f32, tag="px", name=f"po{st}_{nh}")
                                nc.tensor.matmul(
                                    out=state["po"],
                                    lhsT=ct_r[:, c, st * 128:(st + 1) * 128],
                                    rhs=wo_r[:, c, nh * 512:(nh + 1) * 512],
                                    start=(c == 0), stop=(c == NPC - 1),
                                )
                                if c == NPC - 1:
                                    ob = outp.tile([128, 512], f32, tag="ob",
                                                   name=f"ob{st}_{nh}")
                                    nc.vector.tensor_copy(out=ob, in_=state["po"])
                                    eng = nc.sync if st % 2 == 0 else nc.scalar
                                    eng.dma_start(
                                        out=out[st * 128:(st + 1) * 128,
                                                nh * 512:(nh + 1) * 512],
                                        in_=ob)
                            return go
                        for c in range(NPC):
                            filler.append(mk(c))

            def normalize(j, hp, pcs):
                # pcs[hh] rows 64-127 hold the softmax denominator replicated
                # per partition (ones-block matmul output): reciprocal + mul
                # directly from PSUM, all on the DVE.
                for hh in range(2):
                    rc = smal.tile([64, SBK], f32, tag=f"rc{hh}",
                                   name=f"rc{j}{hp}{hh}")
                    nc.vector.reciprocal(out=rc, in_=pcs[hh][64:128, :])
                    nc.vector.tensor_mul(
                        out=ct_r[hh * 64:(hh + 1) * 64, hp, j * SBK:(j + 1) * SBK],
                        in0=pcs[hh][0:64, :],
                        in1=rc,
                    )

            # Software-pipelined driver (depth 2): at step i emit S(i), exp(i),
            # then the PV of step i-2 — whose exp finished two steps ago, so
            # the PE never sits in an exp-wait ahead of the next S and the exp
            # stream stays back-to-back.
            pend = []

            def drain_pv():
                if not pend:
                    return
                j, hp, t, es, pcs = pend.pop(0)
                for hh in range(2):
                    nc.tensor.matmul(
                        out=pcs[hh],
                        lhsT=vaug[:, t, hp * 2 + hh, :],
                        rhs=es[:, hh, :],
                        start=(t == 0), stop=(t == NT - 1),
                    )
                if t == NT - 1:
                    normalize(j, hp, pcs)
                    if hp == 1:
                        queue_outproj(j)

            pcs_by = {}
            for j in range(NSB):
                for hp in range(NPC):
                    pcs_by[(j, hp)] = [
                        ps_c.tile([128, SBK], f32, tag=f"pc{hh}", name=f"pc{hh}_{j}_{hp}")
                        for hh in range(2)]
                    if (j, hp) == (0, 0):
                        for jj in range(NSB):
                            queue_proj_qk(wk_r, bk_sb, kt_r, 1, jj)
                        queue_proj_qk(wq_r, bq_sb, qt_r, 1, 0)
                    if hp == 1 and j + 1 < NSB:
                        for c in range(NPC):
                            queue_proj_qk(wq_r, bq_sb, qt_r, c, j + 1)
                    for t in range(NT):
                        if j == 0 and hp == 0:
                            proj_v(ps_x, t)        # V projection rides along
                        ss = ps_s.tile([128, 2, SBK], f32, tag="ss", name=f"ss{j}_{hp}_{t}")
                        for hh in range(2):
                            nc.tensor.matmul(
                                out=ss[:, hh, :],
                                lhsT=kt_r[hh * 64:(hh + 1) * 64, hp, t * 128:(t + 1) * 128],
                                rhs=qt_r[hh * 64:(hh + 1) * 64, hp, j * SBK:(j + 1) * SBK],
                                start=True, stop=True,
                            )
                        es = esp.tile([128, 2, SBK], bf16, tag="es", name=f"es{j}_{hp}_{t}")
                        nc.scalar.activation(out=es, in_=ss, func=AF.Exp, scale=0.125)
                        if len(pend) >= 2:
                            drain_pv()
                        pend.append((j, hp, t, es, pcs_by[(j, hp)]))
                        feed(3)
            drain_pv()
            drain_pv()
            flush()

    nc.finalize()
    return nc


def _get_program():
    if "nc" not in _prog_cache:
        _prog_cache["nc"] = _build_program()
    return _prog_cache["nc"]


def _make_in_maps(x, Wq, bq, Wk, bk, Wv, bv, Wo, bo):
    in_maps = []
    for c in range(NCORES):
        b, hg = divmod(c, TP)
        sl = slice(hg * DP, (hg + 1) * DP)
        in_maps.append({
            "x": np.ascontiguousarray(x[b]),
            "wq": np.ascontiguousarray(Wq[:, sl]),
            "wk": np.ascontiguousarray(Wk[:, sl]),
            "wv": np.ascontiguousarray(Wv[:, sl]),
            "wo": np.ascontiguousarray(Wo[sl, :]),
            "bq": np.ascontiguousarray(bq[sl]),
            "bk": np.ascontiguousarray(bk[sl]),
            "bv": np.ascontiguousarray(bv[sl]),
        })
    return in_maps


def run(inputs, **spmd_kwargs):
    """Build, run on 8 cores, gather. Returns (output, BassKernelResults)."""
    args = {k: np.asarray(v, dtype=np.float32) for k, v in inputs.items()}
    nc = _get_program()
    in_maps = _make_in_maps(
        args["x"], args["Wq"], args["bq"], args["Wk"], args["bk"],
        args["Wv"], args["bv"], args["Wo"], args["bo"],
    )
    res = run_bass_kernel_spmd(nc, in_maps, list(range(NCORES)), **spmd_kwargs)
    out = np.zeros((B, S, D), dtype=np.float32)
    for c in range(NCORES):
        b = c // TP
        out[b] += res.results[c]["out"]
    out += args["bo"]
    return out, res


def kernel(**inputs):
    out, _ = run(inputs)
    return out



# revision 10
# speedup vs baseline: 1.0928x; 1.0005x over previous
"""Multi-head attention (B=2, S=2048, D=1024, H=16, Dk=64) on 8 NeuronCores.

Sharding: 2-way data parallel over batch x 4-way tensor parallel over heads.
Core c handles batch c//4 and heads (c%4)*4 .. (c%4)*4+3, i.e. a 256-column
slice of the QKV projections and the matching 256-row slice of Wo. Each core
computes a partial output projection [S, D]; the host sums the 4 partials per
batch (the all-reduce of the sharding hint) and stacks the batches.

On-core algorithm (matmuls in float32r = full-rate fp32, PSUM accum fp32):
  x^T via PE transpose -> Q^T, K^T head-packed [128, 2, S] (head parity on
  partition halves 0-63/64-127 so the two heads' K=64 score matmuls run
  concurrently in separate PE row groups) and V in natural [t, d'] layout,
  augmented with a ones column -> S^T = K_h Q_h^T -> exp on ACT (1/8 scale
  folded into the activation; no max subtraction needed: scores are O(5)
  for unit-variance inputs, far from fp32 overflow) -> C^T = V_aug^T @
  expS^T where the ones row yields the softmax denominator for free ->
  normalize -> partial out = C^T.T @ Wo_slice + bo/4.

Emission order interleaves the V projection and the j+1 Q projection into
attention block j so the PE keeps dense work while ACT grinds through the
exp stream; x/output DMAs are split across the SP/ACT HWDGE queues and
weight loads go via the gpsimd SWDGE queues.
"""
from contextlib import ExitStack

import numpy as np
import concourse.bass as bass
import concourse.mybir as mybir
import concourse.tile as tile
from concourse import bacc
from concourse.bass_utils import run_bass_kernel_spmd
from concourse.masks import make_identity

f32 = mybir.dt.float32
f32r = mybir.dt.float32r
bf16 = mybir.dt.bfloat16
AF = mybir.ActivationFunctionType
ALU = mybir.AluOpType

B, S, D = 2, 2048, 1024
H, DK = 16, 64
NCORES = 8
TP = 4                 # tensor-parallel factor (head groups)
HPC = H // TP          # 4 heads per core
DP = HPC * DK          # 256 = per-core d' slice
SBK = 512              # s-block for attention streaming
NSB = S // SBK         # 4
NT = S // 128          # 16 t-tiles
NDC = D // 128         # 8 contraction chunks over D
NPC = DP // 128        # 2 chunks over d'

_prog_cache = {}


def _build_program():
    nc = bacc.Bacc()
    x = nc.dram_tensor("x", [S, D], f32, kind="ExternalInput")
    wq = nc.dram_tensor("wq", [D, DP], f32, kind="ExternalInput")
    wk = nc.dram_tensor("wk", [D, DP], f32, kind="ExternalInput")
    wv = nc.dram_tensor("wv", [D, DP], f32, kind="ExternalInput")
    wo = nc.dram_tensor("wo", [DP, D], f32, kind="ExternalInput")
    bq = nc.dram_tensor("bq", [DP], f32, kind="ExternalInput")
    bk = nc.dram_tensor("bk", [DP], f32, kind="ExternalInput")
    bv = nc.dram_tensor("bv", [DP], f32, kind="ExternalInput")
    out = nc.dram_tensor("out", [S, D], f32, kind="ExternalOutput")

    with tile.TileContext(nc) as tc, ExitStack() as top:
        const = top.enter_context(tc.tile_pool(name="const", bufs=1))
        big = top.enter_context(tc.tile_pool(name="big", bufs=1))
        xtp = top.enter_context(tc.tile_pool(name="xt", bufs=1))

        ident = const.tile([128, 128], f32)
        make_identity(nc, ident)

        # persistent activations
        qt_r = big.tile([128, NPC, S], f32r)
        kt_r = big.tile([128, NPC, S], f32r)
        # V augmented with a 64-wide ones block: the PV matmul then emits the
        # softmax denominator replicated across PSUM partitions 64-127, so
        # normalization is a straight DVE reciprocal+mul from PSUM (no gpsimd
        # transpose/broadcast round-trips).
        vaug = big.tile([128, NT, HPC, 2 * DK], bf16)
        ct_r = big.tile([128, NPC, S], f32r)
        xt_r = xtp.tile([128, NDC, S], bf16)

        wq_r = const.tile([128, NDC, DP], bf16)
        wv_r = const.tile([128, NDC, DP], bf16)
        wo_r = const.tile([128, NPC, D], f32r)
        bq_sb = const.tile([128, NPC], f32)
        bk_sb = const.tile([128, NPC], f32)
        bv_b = const.tile([128, DP], f32)

        wkp = top.enter_context(tc.tile_pool(name="wkp", bufs=1))
        wk_r = wkp.tile([128, NDC, DP], bf16)

        es_ld = ExitStack()
        stg = es_ld.enter_context(tc.tile_pool(name="stg", bufs=2))
        xin = es_ld.enter_context(tc.tile_pool(name="xin", bufs=2))
        ps_t = es_ld.enter_context(tc.tile_pool(name="ps_t", bufs=2, space="PSUM"))

        # ---- loads: x tiles on the two HWDGE queues, weights on SWDGE ----
        x_tiles = []
        for st in range(NT):
            x_t = xin.tile([128, D], f32, tag="x_t", name=f"x_t{st}")
            eng = nc.sync if st % 2 == 0 else nc.scalar
            eng.dma_start(out=x_t, in_=x[st * 128:(st + 1) * 128, :])
            x_tiles.append(x_t)

        wstg = {}
        for src, npc, nm in ((wq, NDC, "wq"), (wk, NDC, "wk"),
                             (wv, NDC, "wv"), (wo, NPC, "wo")):
            sf = stg.tile([128, npc, src.shape[1]], f32, tag="wstg", name=f"stg_{nm}")
            nc.gpsimd.dma_start(out=sf, in_=src.rearrange("(ko ki) d -> ki ko d", ki=128))
            wstg[nm] = sf
        nc.gpsimd.dma_start(out=bq_sb, in_=bq[:].rearrange("(c p) -> p c", p=128))
        nc.gpsimd.dma_start(out=bk_sb, in_=bk[:].rearrange("(c p) -> p c", p=128))
        bv_1 = const.tile([1, DP], f32)
        nc.gpsimd.dma_start(out=bv_1, in_=bv[:].rearrange("(a d) -> a d", a=1))
        nc.gpsimd.partition_broadcast(bv_b, bv_1)

        nc.vector.tensor_copy(out=wq_r, in_=wstg["wq"])
        nc.vector.tensor_copy(out=wk_r, in_=wstg["wk"])

        def proj_qk(pool, wr, bias_sb, dst, c, j):
            pq = pool.tile([128, SBK], f32, tag="pqkv", name=f"pj{c}_{j}_{id(wr)%89}")
            for k in range(NDC):
                nc.tensor.matmul(
                    out=pq,
                    lhsT=wr[:, k, c * 128:(c + 1) * 128],
                    rhs=xt_r[:, k, j * SBK:(j + 1) * SBK],
                    start=(k == 0), stop=(k == NDC - 1),
                )
            nc.vector.tensor_scalar_add(
                out=dst[:, c, j * SBK:(j + 1) * SBK],
                in0=pq, scalar1=bias_sb[:, c:c + 1],
            )

        def proj_v(pool, st):
            pv = pool.tile([128, DP], f32, tag="px", name=f"pv{st}")
            for k in range(NDC):
                nc.tensor.matmul(
                    out=pv,
                    lhsT=xt_r[:, k, st * 128:(st + 1) * 128],
                    rhs=wv_r[:, k, :],
                    start=(k == 0), stop=(k == NDC - 1),
                )
            nc.vector.tensor_add(
                out=vaug[:, st, :, 0:DK],
                in0=pv.rearrange("p (h d) -> p h d", h=HPC),
                in1=bv_b.rearrange("p (h d) -> p h d", h=HPC),
            )

        # ---- x^T via PE transpose, interleaved with the c=0 K projections
        # so the first attention block can start as early as possible ----
        ps_p = es_ld.enter_context(tc.tile_pool(name="ps_p", bufs=2, space="PSUM"))
        for g in range(4):
            for st in range(4 * g, 4 * g + 4):
                tp = ps_t.tile([128, NDC * 128], f32, tag="tp", name=f"tp{st}")
                for k in range(NDC):
                    nc.tensor.transpose(
                        out=tp[:, k * 128:(k + 1) * 128],
                        in_=x_tiles[st][:, k * 128:(k + 1) * 128],
                        identity=ident,
                    )
                nc.vector.tensor_copy(
                    out=xt_r[:, :, st * 128:(st + 1) * 128],
                    in_=tp.rearrange("p (k s) -> p k s", k=NDC),
                )
            proj_qk(ps_p, wk_r, bk_sb, kt_r, 0, g)
        proj_qk(ps_p, wq_r, bq_sb, qt_r, 0, 0)

        nc.vector.tensor_copy(out=wv_r, in_=wstg["wv"])
        nc.vector.tensor_copy(out=wo_r, in_=wstg["wo"])
        nc.vector.memset(vaug[:, :, :, DK:2 * DK], 1.0)

        es_ld.close()   # frees stg + xin SBUF and the phase-1 PSUM banks

        # ---- attention + output projection ----
        with ExitStack() as ph2:
            esp = ph2.enter_context(tc.tile_pool(name="esp", bufs=3))
            smal = ph2.enter_context(tc.tile_pool(name="smal", bufs=2))
            outp = ph2.enter_context(tc.tile_pool(name="outp", bufs=2))
            ps_x = ph2.enter_context(tc.tile_pool(name="ps_x", bufs=2, space="PSUM"))
            ps_s = ph2.enter_context(tc.tile_pool(name="ps_s", bufs=2, space="PSUM"))
            ps_c = ph2.enter_context(tc.tile_pool(name="ps_c", bufs=1, space="PSUM"))

            # Filler queue: exp-independent PE work (out-projections, next-j Q
            # projections) dripped into the attention t-loop so the PE never
            # idles waiting on ACT and the HAM clock stays warm.
            filler = []

            def feed(n):
                for _ in range(min(n, len(filler))):
                    filler.pop(0)()

            def flush():
                while filler:
                    filler.pop(0)()

            def queue_proj_qk(wr, bias_sb, dst, c, j):
                state = {}
                def mk(k):
                    def go():
                        if k == 0:
                            state["pq"] = ps_x.tile(
                                [128, SBK], f32, tag="px", name=f"fq{c}_{j}_{k}")
                        nc.tensor.matmul(
                            out=state["pq"],
                            lhsT=wr[:, k, c * 128:(c + 1) * 128],
                            rhs=xt_r[:, k, j * SBK:(j + 1) * SBK],
                            start=(k == 0), stop=(k == NDC - 1),
                        )
                        if k == NDC - 1:
                            nc.vector.tensor_scalar_add(
                                out=dst[:, c, j * SBK:(j + 1) * SBK],
                                in0=state["pq"], scalar1=bias_sb[:, c:c + 1],
                            )
                    return go
                for k in range(NDC):
                    filler.append(mk(k))

            def queue_outproj(j):
                for stj in range(SBK // 128):
                    st = j * (SBK // 128) + stj
                    for nh in range(2):
                        state = {}
                        def mk(c, st=st, nh=nh, state=state):
                            def go():
                                if c == 0:
                                    state["po"] = ps_x.tile(
                                        [128, 512], f32, tag="px", name=f"po{st}_{nh}")
                                nc.tensor.matmul(
                                    out=state["po"],
                                    lhsT=ct_r[:, c, st * 128:(st + 1) * 128],
                                    rhs=wo_r[:, c, nh * 512:(nh + 1) * 512],
                                    start=(c == 0), stop=(c == NPC - 1),
                                )
                                if c == NPC - 1:
                                    ob = outp.tile([128, 512], f32, tag="ob",
                                                   name=f"ob{st}_{nh}")
                                    nc.vector.tensor_copy(out=ob, in_=state["po"])
                                    eng = nc.sync if st % 2 == 0 else nc.scalar
                                    eng.dma_start(
                                        out=out[st * 128:(st + 1) * 128,
                                                nh * 512:(nh + 1) * 512],
                                        in_=ob)
                            return go
                        for c in range(NPC):
                            filler.append(mk(c))

            def normalize(j, hp, pcs):
                # pcs[hh] rows 64-127 hold the softmax denominator replicated
                # per partition (ones-block matmul output): reciprocal + mul
                # directly from PSUM, all on the DVE.
                for hh in range(2):
                    rc = smal.tile([64, SBK], f32, tag=f"rc{hh}",
                                   name=f"rc{j}{hp}{hh}")
                    nc.vector.reciprocal(out=rc, in_=pcs[hh][64:128, :])
                    nc.vector.tensor_mul(
                        out=ct_r[hh * 64:(hh + 1) * 64, hp, j * SBK:(j + 1) * SBK],
                        in0=pcs[hh][0:64, :],
                        in1=rc,
                    )

            # Software-pipelined driver (depth 2): at step i emit S(i), exp(i),
            # then the PV of step i-2 — whose exp finished two steps ago, so
            # the PE never sits in an exp-wait ahead of the next S and the exp
            # stream stays back-to-back.
            pend = []

            def drain_pv():
                if not pend:
                    return
                j, hp, t, es, pcs = pend.pop(0)
                for hh in range(2):
                    nc.tensor.matmul(
                        out=pcs[hh],
                        lhsT=vaug[:, t, hp * 2 + hh, :],
                        rhs=es[:, hh, :],
                        start=(t == 0), stop=(t == NT - 1),
                    )
                if t == NT - 1:
                    normalize(j, hp, pcs)
                    if hp == 1:
                        queue_outproj(j)

            pcs_by = {}
            for j in range(NSB):
                for hp in range(NPC):
                    pcs_by[(j, hp)] = [
                        ps_c.tile([128, SBK], f32, tag=f"pc{hh}", name=f"pc{hh}_{j}_{hp}")
                        for hh in range(2)]
                    if (j, hp) == (0, 0):
                        for jj in range(NSB):
                            queue_proj_qk(wk_r, bk_sb, kt_r, 1, jj)
                        queue_proj_qk(wq_r, bq_sb, qt_r, 1, 0)
                    if hp == 1 and j + 1 < NSB:
                        for c in range(NPC):
                            queue_proj_qk(wq_r, bq_sb, qt_r, c, j + 1)
                    for t in range(NT):
                        if j == 0 and hp == 0:
                            proj_v(ps_x, t)        # V projection rides along
                        ss = ps_s.tile([128, 2, SBK], f32, tag="ss", name=f"ss{j}_{hp}_{t}")
                        for hh in range(2):
                            nc.tensor.matmul(
                                out=ss[:, hh, :],
                                lhsT=kt_r[hh * 64:(hh + 1) * 64, hp, t * 128:(t + 1) * 128],
                                rhs=qt_r[hh * 64:(hh + 1) * 64, hp, j * SBK:(j + 1) * SBK],
                                start=True, stop=True,
                            )
                        es = esp.tile([128, 2, SBK], bf16, tag="es", name=f"es{j}_{hp}_{t}")
                        nc.scalar.activation(out=es, in_=ss, func=AF.Exp, scale=0.125)
                        if len(pend) >= 2:
                            drain_pv()
                        pend.append((j, hp, t, es, pcs_by[(j, hp)]))
                        feed(3)
            drain_pv()
            drain_pv()
            flush()

    nc.finalize()
    return nc


def _get_program():
    if "nc" not in _prog_cache:
        _prog_cache["nc"] = _build_program()
    return _prog_cache["nc"]


def _make_in_maps(x, Wq, bq, Wk, bk, Wv, bv, Wo, bo):
    in_maps = []
    for c in range(NCORES):
        b, hg = divmod(c, TP)
        sl = slice(hg * DP, (hg + 1) * DP)
        in_maps.append({
            "x": np.ascontiguousarray(x[b]),
            "wq": np.ascontiguousarray(Wq[:, sl]),
            "wk": np.ascontiguousarray(Wk[:, sl]),
            "wv": np.ascontiguousarray(Wv[:, sl]),
            "wo": np.ascontiguousarray(Wo[sl, :]),
            "bq": np.ascontiguousarray(bq[sl]),
            "bk": np.ascontiguousarray(bk[sl]),
            "bv": np.ascontiguousarray(bv[sl]),
        })
    return in_maps


def run(inputs, **spmd_kwargs):
    """Build, run on 8 cores, gather. Returns (output, BassKernelResults)."""
    args = {k: np.asarray(v, dtype=np.float32) for k, v in inputs.items()}
    nc = _get_program()
    in_maps = _make_in_maps(
        args["x"], args["Wq"], args["bq"], args["Wk"], args["bk"],
        args["Wv"], args["bv"], args["Wo"], args["bo"],
    )
    res = run_bass_kernel_spmd(nc, in_maps, list(range(NCORES)), **spmd_kwargs)
    out = np.zeros((B, S, D), dtype=np.float32)
    for c in range(NCORES):
        b = c // TP
        out[b] += res.results[c]["out"]
    out += args["bo"]
    return out, res


def kernel(**inputs):
    out, _ = run(inputs)
    return out



# revision 15
# speedup vs baseline: 1.1152x; 1.0205x over previous
"""Multi-head attention (B=2, S=2048, D=1024, H=16, Dk=64) on 8 NeuronCores.

Sharding: 2-way data parallel over batch x 4-way tensor parallel over heads.
Core c handles batch c//4 and heads (c%4)*4 .. (c%4)*4+3, i.e. a 256-column
slice of the QKV projections and the matching 256-row slice of Wo. Each core
computes a partial output projection [S, D]; the host sums the 4 partials per
batch (the all-reduce of the sharding hint) and stacks the batches.

On-core algorithm (matmuls in float32r = full-rate fp32, PSUM accum fp32):
  x^T via PE transpose -> Q^T, K^T head-packed [128, 2, S] (head parity on
  partition halves 0-63/64-127 so the two heads' K=64 score matmuls run
  concurrently in separate PE row groups) and V in natural [t, d'] layout,
  augmented with a ones column -> S^T = K_h Q_h^T -> exp on ACT (1/8 scale
  folded into the activation; no max subtraction needed: scores are O(5)
  for unit-variance inputs, far from fp32 overflow) -> C^T = V_aug^T @
  expS^T where the ones row yields the softmax denominator for free ->
  normalize -> partial out = C^T.T @ Wo_slice + bo/4.

Emission order interleaves the V projection and the j+1 Q projection into
attention block j so the PE keeps dense work while ACT grinds through the
exp stream; x/output DMAs are split across the SP/ACT HWDGE queues and
weight loads go via the gpsimd SWDGE queues.
"""
from contextlib import ExitStack

import numpy as np
import concourse.bass as bass
import concourse.mybir as mybir
import concourse.tile as tile
from concourse import bacc
from concourse.bass_utils import run_bass_kernel_spmd
from concourse.masks import make_identity

f32 = mybir.dt.float32
f32r = mybir.dt.float32r
bf16 = mybir.dt.bfloat16
AF = mybir.ActivationFunctionType
ALU = mybir.AluOpType

B, S, D = 2, 2048, 1024
H, DK = 16, 64
NCORES = 8
TP = 4                 # tensor-parallel factor (head groups)
HPC = H // TP          # 4 heads per core
DP = HPC * DK          # 256 = per-core d' slice
SBK = 512              # s-block for attention streaming
NSB = S // SBK         # 4
NT = S // 128          # 16 t-tiles
NDC = D // 128         # 8 contraction chunks over D
NPC = DP // 128        # 2 chunks over d'

_prog_cache = {}


def _build_program():
    nc = bacc.Bacc()
    x = nc.dram_tensor("x", [S, D], f32, kind="ExternalInput")
    wq = nc.dram_tensor("wq", [D, DP], f32, kind="ExternalInput")
    wk = nc.dram_tensor("wk", [D, DP], f32, kind="ExternalInput")
    wv = nc.dram_tensor("wv", [D, DP], f32, kind="ExternalInput")
    wo = nc.dram_tensor("wo", [DP, D], f32, kind="ExternalInput")
    bq = nc.dram_tensor("bq", [DP], f32, kind="ExternalInput")
    bk = nc.dram_tensor("bk", [DP], f32, kind="ExternalInput")
    bv = nc.dram_tensor("bv", [DP], f32, kind="ExternalInput")
    out = nc.dram_tensor("out", [S, D], f32, kind="ExternalOutput")

    with tile.TileContext(nc) as tc, ExitStack() as top:
        const = top.enter_context(tc.tile_pool(name="const", bufs=1))
        big = top.enter_context(tc.tile_pool(name="big", bufs=1))
        xtp = top.enter_context(tc.tile_pool(name="xt", bufs=1))

        ident = const.tile([128, 128], f32)
        make_identity(nc, ident)

        # persistent activations
        qt_r = big.tile([128, NPC, S], bf16)
        kt_r = big.tile([128, NPC, S], bf16)
        # V augmented with a 64-wide ones block: the PV matmul then emits the
        # softmax denominator replicated across PSUM partitions 64-127, so
        # normalization is a straight DVE reciprocal+mul from PSUM (no gpsimd
        # transpose/broadcast round-trips).
        vaug = big.tile([128, NT, HPC, 2 * DK], bf16)
        ct_r = big.tile([128, NPC, S], bf16)
        xt_r = xtp.tile([128, NDC, S], bf16)

        wq_r = const.tile([128, NDC, DP], bf16)
        wv_r = const.tile([128, NDC, DP], bf16)
        wo_r = const.tile([128, NPC, D], bf16)
        bq_sb = const.tile([128, NPC], f32)
        bk_sb = const.tile([128, NPC], f32)
        bv_b = const.tile([128, DP], f32)

        wkp = top.enter_context(tc.tile_pool(name="wkp", bufs=1))
        wk_r = wkp.tile([128, NDC, DP], bf16)

        es_ld = ExitStack()
        stg = es_ld.enter_context(tc.tile_pool(name="stg", bufs=2))
        xin = es_ld.enter_context(tc.tile_pool(name="xin", bufs=2))
        ps_t = es_ld.enter_context(tc.tile_pool(name="ps_t", bufs=2, space="PSUM"))

        # ---- loads: x tiles on the two HWDGE queues, weights on SWDGE ----
        x_tiles = []
        for st in range(NT):
            x_t = xin.tile([128, D], f32, tag="x_t", name=f"x_t{st}")
            eng = nc.sync if st % 2 == 0 else nc.scalar
            eng.dma_start(out=x_t, in_=x[st * 128:(st + 1) * 128, :])
            x_tiles.append(x_t)

        wstg = {}
        for src, npc, nm in ((wq, NDC, "wq"), (wk, NDC, "wk"),
                             (wv, NDC, "wv"), (wo, NPC, "wo")):
            sf = stg.tile([128, npc, src.shape[1]], f32, tag="wstg", name=f"stg_{nm}")
            nc.gpsimd.dma_start(out=sf, in_=src.rearrange("(ko ki) d -> ki ko d", ki=128))
            wstg[nm] = sf
        nc.gpsimd.dma_start(out=bq_sb, in_=bq[:].rearrange("(c p) -> p c", p=128))
        nc.gpsimd.dma_start(out=bk_sb, in_=bk[:].rearrange("(c p) -> p c", p=128))
        bv_1 = const.tile([1, DP], f32)
        nc.gpsimd.dma_start(out=bv_1, in_=bv[:].rearrange("(a d) -> a d", a=1))
        nc.gpsimd.partition_broadcast(bv_b, bv_1)

        nc.vector.tensor_copy(out=wq_r, in_=wstg["wq"])
        nc.vector.tensor_copy(out=wk_r, in_=wstg["wk"])

        def proj_qk(pool, wr, bias_sb, dst, c, j):
            pq = pool.tile([128, SBK], f32, tag="pqkv", name=f"pj{c}_{j}_{id(wr)%89}")
            for k in range(NDC):
                nc.tensor.matmul(
                    out=pq,
                    lhsT=wr[:, k, c * 128:(c + 1) * 128],
                    rhs=xt_r[:, k, j * SBK:(j + 1) * SBK],
                    start=(k == 0), stop=(k == NDC - 1),
                )
            nc.vector.tensor_scalar_add(
                out=dst[:, c, j * SBK:(j + 1) * SBK],
                in0=pq, scalar1=bias_sb[:, c:c + 1],
            )

        def proj_v(pool, st):
            pv = pool.tile([128, DP], f32, tag="px", name=f"pv{st}")
            for k in range(NDC):
                nc.tensor.matmul(
                    out=pv,
                    lhsT=xt_r[:, k, st * 128:(st + 1) * 128],
                    rhs=wv_r[:, k, :],
                    start=(k == 0), stop=(k == NDC - 1),
                )
            nc.vector.tensor_add(
                out=vaug[:, st, :, 0:DK],
                in0=pv.rearrange("p (h d) -> p h d", h=HPC),
                in1=bv_b.rearrange("p (h d) -> p h d", h=HPC),
            )

        # ---- x^T via PE transpose, interleaved with the c=0 K projections
        # so the first attention block can start as early as possible ----
        ps_p = es_ld.enter_context(tc.tile_pool(name="ps_p", bufs=2, space="PSUM"))
        for g in range(4):
            for st in range(4 * g, 4 * g + 4):
                tp = ps_t.tile([128, NDC * 128], f32, tag="tp", name=f"tp{st}")
                for k in range(NDC):
                    nc.tensor.transpose(
                        out=tp[:, k * 128:(k + 1) * 128],
                        in_=x_tiles[st][:, k * 128:(k + 1) * 128],
                        identity=ident,
                    )
                nc.vector.tensor_copy(
                    out=xt_r[:, :, st * 128:(st + 1) * 128],
                    in_=tp.rearrange("p (k s) -> p k s", k=NDC),
                )
            proj_qk(ps_p, wk_r, bk_sb, kt_r, 0, g)
        proj_qk(ps_p, wq_r, bq_sb, qt_r, 0, 0)

        nc.vector.tensor_copy(out=wv_r, in_=wstg["wv"])
        nc.vector.tensor_copy(out=wo_r, in_=wstg["wo"])
        nc.vector.memset(vaug[:, :, :, DK:2 * DK], 1.0)

        es_ld.close()   # frees stg + xin SBUF and the phase-1 PSUM banks

        # ---- attention + output projection ----
        with ExitStack() as ph2:
            esp = ph2.enter_context(tc.tile_pool(name="esp", bufs=3))
            smal = ph2.enter_context(tc.tile_pool(name="smal", bufs=2))
            outp = ph2.enter_context(tc.tile_pool(name="outp", bufs=2))
            ps_x = ph2.enter_context(tc.tile_pool(name="ps_x", bufs=2, space="PSUM"))
            ps_s = ph2.enter_context(tc.tile_pool(name="ps_s", bufs=2, space="PSUM"))
            ps_c = ph2.enter_context(tc.tile_pool(name="ps_c", bufs=1, space="PSUM"))

            # Filler queue: exp-independent PE work (out-projections, next-j Q
            # projections) dripped into the attention t-loop so the PE never
            # idles waiting on ACT and the HAM clock stays warm.
            filler = []

            def feed(n):
                for _ in range(min(n, len(filler))):
                    filler.pop(0)()

            def flush():
                while filler:
                    filler.pop(0)()

            def queue_proj_qk(wr, bias_sb, dst, c, j):
                state = {}
                def mk(k):
                    def go():
                        if k == 0:
                            state["pq"] = ps_x.tile(
                                [128, SBK], f32, tag="px", name=f"fq{c}_{j}_{k}")
                        nc.tensor.matmul(
                            out=state["pq"],
                            lhsT=wr[:, k, c * 128:(c + 1) * 128],
                            rhs=xt_r[:, k, j * SBK:(j + 1) * SBK],
                            start=(k == 0), stop=(k == NDC - 1),
                        )
                        if k == NDC - 1:
                            nc.vector.tensor_scalar_add(
                                out=dst[:, c, j * SBK:(j + 1) * SBK],
                                in0=state["pq"], scalar1=bias_sb[:, c:c + 1],
                            )
                    return go
                for k in range(NDC):
                    filler.append(mk(k))

            def queue_outproj(j):
                for stj in range(SBK // 128):
                    st = j * (SBK // 128) + stj
                    for nh in range(2):
                        state = {}
                        def mk(c, st=st, nh=nh, state=state):
                            def go():
                                if c == 0:
                                    state["po"] = ps_x.tile(
                                        [128, 512], f32, tag="px", name=f"po{st}_{nh}")
                                nc.tensor.matmul(
                                    out=state["po"],
                                    lhsT=ct_r[:, c, st * 128:(st + 1) * 128],
                                    rhs=wo_r[:, c, nh * 512:(nh + 1) * 512],
                                    start=(c == 0), stop=(c == NPC - 1),
                                )
                                if c == NPC - 1:
                                    ob = outp.tile([128, 512], f32, tag="ob",
                                                   name=f"ob{st}_{nh}")
                                    nc.vector.tensor_copy(out=ob, in_=state["po"])
                                    eng = nc.sync if st % 2 == 0 else nc.scalar
                                    eng.dma_start(
                                        out=out[st * 128:(st + 1) * 128,
                                                nh * 512:(nh + 1) * 512],
                                        in_=ob)
                            return go
                        for c in range(NPC):
                            filler.append(mk(c))

            def normalize(j, hp, pcs):
                # pcs[hh] rows 64-127 hold the softmax denominator replicated
                # per partition (ones-block matmul output): approx-reciprocal
                # (~5x faster than reciprocal(), 18-bit accurate) + mul, both
                # DVE, reading straight from PSUM. (Full reciprocal() is ~12
                # cyc/elem and stalled the PE at every phase boundary; DVE
                # divide doesn't compile on HW.)
                for hh in range(2):
                    dn = smal.tile([64, SBK], f32, tag=f"dn{hh}",
                                   name=f"dn{j}{hp}{hh}")
                    nc.vector.tensor_copy(out=dn, in_=pcs[hh][64:128, :])
                    rc = smal.tile([64, SBK], f32, tag=f"rc{hh}",
                                   name=f"rc{j}{hp}{hh}")
                    nc.vector.reciprocal_approx_fast(out=rc, in_=dn)
                    nc.vector.tensor_mul(
                        out=ct_r[hh * 64:(hh + 1) * 64, hp, j * SBK:(j + 1) * SBK],
                        in0=pcs[hh][0:64, :],
                        in1=rc,
                    )

            # Software-pipelined driver (depth 2): at step i emit S(i), exp(i),
            # then the PV of step i-2 — whose exp finished two steps ago, so
            # the PE never sits in an exp-wait ahead of the next S and the exp
            # stream stays back-to-back.
            pend = []

            def drain_pv():
                if not pend:
                    return
                j, hp, t, es, pcs = pend.pop(0)
                for hh in range(2):
                    nc.tensor.matmul(
                        out=pcs[hh],
                        lhsT=vaug[:, t, hp * 2 + hh, :],
                        rhs=es[:, hh, :],
                        start=(t == 0), stop=(t == NT - 1),
                    )
                if t == NT - 1:
                    normalize(j, hp, pcs)
                    if hp == 1:
                        queue_outproj(j)

            pcs_by = {}
            for j in range(NSB):
                for hp in range(NPC):
                    pcs_by[(j, hp)] = [
                        ps_c.tile([128, SBK], f32, tag=f"pc{hh}", name=f"pc{hh}_{j}_{hp}")
                        for hh in range(2)]
                    if (j, hp) == (0, 0):
                        for jj in range(NSB):
                            queue_proj_qk(wk_r, bk_sb, kt_r, 1, jj)
                        queue_proj_qk(wq_r, bq_sb, qt_r, 1, 0)
                    if hp == 1 and j + 1 < NSB:
                        for c in range(NPC):
                            queue_proj_qk(wq_r, bq_sb, qt_r, c, j + 1)
                    for t in range(NT):
                        if j == 0 and hp == 0:
                            proj_v(ps_x, t)        # V projection rides along
                        ss = ps_s.tile([128, 2, SBK], f32, tag="ss", name=f"ss{j}_{hp}_{t}")
                        for hh in range(2):
                            nc.tensor.matmul(
                                out=ss[:, hh, :],
                                lhsT=kt_r[hh * 64:(hh + 1) * 64, hp, t * 128:(t + 1) * 128],
                                rhs=qt_r[hh * 64:(hh + 1) * 64, hp, j * SBK:(j + 1) * SBK],
                                start=True, stop=True,
                            )
                        es = esp.tile([128, 2, SBK], bf16, tag="es", name=f"es{j}_{hp}_{t}")
                        nc.scalar.activation(out=es, in_=ss, func=AF.Exp, scale=0.125)
                        if len(pend) >= 2:
                            drain_pv()
                        pend.append((j, hp, t, es, pcs_by[(j, hp)]))
                        feed(3)
            drain_pv()
            drain_pv()
            flush()

    nc.finalize()
    return nc


def _get_program():
    if "nc" not in _prog_cache:
        _prog_cache["nc"] = _build_program()
    return _prog_cache["nc"]


def _make_in_maps(x, Wq, bq, Wk, bk, Wv, bv, Wo, bo):
    in_maps = []
    for c in range(NCORES):
        b, hg = divmod(c, TP)
        sl = slice(hg * DP, (hg + 1) * DP)
        in_maps.append({
            "x": np.ascontiguousarray(x[b]),
            "wq": np.ascontiguousarray(Wq[:, sl]),
            "wk": np.ascontiguousarray(Wk[:, sl]),
            "wv": np.ascontiguousarray(Wv[:, sl]),
            "wo": np.ascontiguousarray(Wo[sl, :]),
            "bq": np.ascontiguousarray(bq[sl]),
            "bk": np.ascontiguousarray(bk[sl]),
            "bv": np.ascontiguousarray(bv[sl]),
        })
    return in_maps


def run(inputs, **spmd_kwargs):
    """Build, run on 8 cores, gather. Returns (output, BassKernelResults)."""
    args = {k: np.asarray(v, dtype=np.float32) for k, v in inputs.items()}
    nc = _get_program()
    in_maps = _make_in_maps(
        args["x"], args["Wq"], args["bq"], args["Wk"], args["bk"],
        args["Wv"], args["bv"], args["Wo"], args["bo"],
    )
    res = run_bass_kernel_spmd(nc, in_maps, list(range(NCORES)), **spmd_kwargs)
    out = np.zeros((B, S, D), dtype=np.float32)
    for c in range(NCORES):
        b = c // TP
        out[b] += res.results[c]["out"]
    out += args["bo"]
    return out, res


def kernel(**inputs):
    out, _ = run(inputs)
    return out



# revision 22
# speedup vs baseline: 1.4443x; 1.2952x over previous
"""Multi-head attention (B=2, S=2048, D=1024, H=16, Dk=64) on 8 NeuronCores.

Sharding: 2-way data parallel over batch x 4-way tensor parallel over heads.
Core c handles batch c//4 and heads (c%4)*4 .. (c%4)*4+3, i.e. a 256-column
slice of the QKV projections and the matching 256-row slice of Wo. Each core
computes a partial output projection [S, D]; the host sums the 4 partials per
batch (the all-reduce of the sharding hint) and stacks the batches.

On-core algorithm (matmuls in float32r = full-rate fp32, PSUM accum fp32):
  x^T via PE transpose -> Q^T, K^T head-packed [128, 2, S] (head parity on
  partition halves 0-63/64-127 so the two heads' K=64 score matmuls run
  concurrently in separate PE row groups) and V in natural [t, d'] layout,
  augmented with a ones column -> S^T = K_h Q_h^T -> exp on ACT (1/8 scale
  folded into the activation; no max subtraction needed: scores are O(5)
  for unit-variance inputs, far from fp32 overflow) -> C^T = V_aug^T @
  expS^T where the ones row yields the softmax denominator for free ->
  normalize -> partial out = C^T.T @ Wo_slice + bo/4.

Emission order interleaves the V projection and the j+1 Q projection into
attention block j so the PE keeps dense work while ACT grinds through the
exp stream; x/output DMAs are split across the SP/ACT HWDGE queues and
weight loads go via the gpsimd SWDGE queues.
"""
from contextlib import ExitStack

import numpy as np
import concourse.bass as bass
import concourse.mybir as mybir
import concourse.tile as tile
from concourse import bacc
from concourse.bass_utils import run_bass_kernel_spmd
from concourse.masks import make_identity

f32 = mybir.dt.float32
f32r = mybir.dt.float32r
bf16 = mybir.dt.bfloat16
AF = mybir.ActivationFunctionType
ALU = mybir.AluOpType

B, S, D = 2, 2048, 1024
H, DK = 16, 64
NCORES = 8
TP = 4                 # tensor-parallel factor (head groups)
HPC = H // TP          # 4 heads per core
DP = HPC * DK          # 256 = per-core d' slice
SBK = 512              # s-block for attention streaming
NSB = S // SBK         # 4
NT = S // 128          # 16 t-tiles
NDC = D // 128         # 8 contraction chunks over D
NPC = DP // 128        # 2 chunks over d'

_prog_cache = {}


def _build_program():
    nc = bacc.Bacc()
    x = nc.dram_tensor("x", [S, D], f32, kind="ExternalInput")
    wq = nc.dram_tensor("wq", [D, DP], f32, kind="ExternalInput")
    wk = nc.dram_tensor("wk", [D, DP], f32, kind="ExternalInput")
    wv = nc.dram_tensor("wv", [D, DP], f32, kind="ExternalInput")
    wo = nc.dram_tensor("wo", [DP, D], f32, kind="ExternalInput")
    bq = nc.dram_tensor("bq", [DP], f32, kind="ExternalInput")
    bk = nc.dram_tensor("bk", [DP], f32, kind="ExternalInput")
    bv = nc.dram_tensor("bv", [DP], f32, kind="ExternalInput")
    out = nc.dram_tensor("out", [S, D], f32, kind="ExternalOutput")

    with tile.TileContext(nc) as tc, ExitStack() as top:
        const = top.enter_context(tc.tile_pool(name="const", bufs=1))
        big = top.enter_context(tc.tile_pool(name="big", bufs=1))
        xtp = top.enter_context(tc.tile_pool(name="xt", bufs=1))

        ident = const.tile([128, 128], f32)
        make_identity(nc, ident)

        # persistent activations
        qt_r = big.tile([128, NPC, S], bf16)
        kt_r = big.tile([128, NPC, S], bf16)
        # V augmented with a 64-wide ones block: the PV matmul then emits the
        # softmax denominator replicated across PSUM partitions 64-127, so
        # normalization is a straight DVE reciprocal+mul from PSUM (no gpsimd
        # transpose/broadcast round-trips).
        vaug = big.tile([128, NT, HPC, 2 * DK], bf16)
        ct_r = big.tile([128, NPC, S], bf16)
        xt_r = xtp.tile([128, NDC, S], bf16)

        wq_r = const.tile([128, NDC, DP], bf16)
        wv_r = const.tile([128, NDC, DP], bf16)
        wo_r = const.tile([128, NPC, D], bf16)
        bq_sb = const.tile([128, NPC], f32)
        bk_sb = const.tile([128, NPC], f32)
        bv_b = const.tile([128, DP], f32)

        wkp = top.enter_context(tc.tile_pool(name="wkp", bufs=1))
        wk_r = wkp.tile([128, NDC, DP], bf16)

        # attention-phase pools (created before the fold pools so the fold
        # stack can be released mid-loop in LIFO order)
        esp = top.enter_context(tc.tile_pool(name="esp", bufs=3))
        smal = top.enter_context(tc.tile_pool(name="smal", bufs=2))
        outp = top.enter_context(tc.tile_pool(name="outp", bufs=2))
        ps_x = top.enter_context(tc.tile_pool(name="ps_x", bufs=2, space="PSUM"))
        ps_c = top.enter_context(tc.tile_pool(name="ps_c", bufs=1, space="PSUM"))
        pools = {}   # ps_s (bufs=2) is created once the fold pools close

        # Fold-phase pools: startup (x transposes, c=0 K/Q projections) is
        # folded into the first attention phase (0,0) so the exp stream and
        # the PE pipeline start ~70us earlier. These close at phase (0,1).
        fold = ExitStack()
        stg = fold.enter_context(tc.tile_pool(name="stg", bufs=2))
        xin = fold.enter_context(tc.tile_pool(name="xin", bufs=2))
        tp_p = fold.enter_context(tc.tile_pool(name="tp", bufs=2, space="PSUM"))
        ss0_p = fold.enter_context(tc.tile_pool(name="ss0", bufs=1, space="PSUM"))

        # ---- loads: x tiles on the two HWDGE queues, weights on SWDGE ----
        x_tiles = []
        for st in range(NT):
            x_t = xin.tile([128, D], f32, tag="x_t", name=f"x_t{st}")
            eng = nc.sync if st % 2 == 0 else nc.scalar
            eng.dma_start(out=x_t, in_=x[st * 128:(st + 1) * 128, :])
            x_tiles.append(x_t)

        # biases first on the SWDGE queue (tiny; the early K bias-add needs
        # bk_sb at ~8us), then weights in first-use order.
        nc.gpsimd.dma_start(out=bq_sb, in_=bq[:].rearrange("(c p) -> p c", p=128))
        nc.gpsimd.dma_start(out=bk_sb, in_=bk[:].rearrange("(c p) -> p c", p=128))
        bv_1 = const.tile([1, DP], f32)
        nc.gpsimd.dma_start(out=bv_1, in_=bv[:].rearrange("(a d) -> a d", a=1))
        nc.gpsimd.partition_broadcast(bv_b, bv_1)
        wstg = {}
        for src, npc, nm in ((wk, NDC, "wk"), (wq, NDC, "wq"),
                             (wv, NDC, "wv"), (wo, NPC, "wo")):
            sf = stg.tile([128, npc, src.shape[1]], f32, tag="wstg", name=f"stg_{nm}")
            nc.gpsimd.dma_start(out=sf, in_=src.rearrange("(ko ki) d -> ki ko d", ki=128))
            wstg[nm] = sf

        nc.vector.tensor_copy(out=wk_r, in_=wstg["wk"])
        nc.vector.tensor_copy(out=wq_r, in_=wstg["wq"])
        nc.vector.tensor_copy(out=wv_r, in_=wstg["wv"])
        nc.vector.memset(vaug[:, :, :, DK:2 * DK], 1.0)

        def proj_qk(pool, wr, bias_sb, dst, c, j):
            pq = pool.tile([128, SBK], f32, tag="px", name=f"pj{c}_{j}_{id(wr)%89}")
            for k in range(NDC):
                nc.tensor.matmul(
                    out=pq,
                    lhsT=wr[:, k, c * 128:(c + 1) * 128],
                    rhs=xt_r[:, k, j * SBK:(j + 1) * SBK],
                    start=(k == 0), stop=(k == NDC - 1),
                )
            nc.vector.tensor_scalar_add(
                out=dst[:, c, j * SBK:(j + 1) * SBK],
                in0=pq, scalar1=bias_sb[:, c:c + 1],
            )

        def proj_v(pool, st):
            pv = pool.tile([128, DP], f32, tag="px", name=f"pv{st}")
            for k in range(NDC):
                nc.tensor.matmul(
                    out=pv,
                    lhsT=xt_r[:, k, st * 128:(st + 1) * 128],
                    rhs=wv_r[:, k, :],
                    start=(k == 0), stop=(k == NDC - 1),
                )
            nc.vector.tensor_add(
                out=vaug[:, st, :, 0:DK],
                in0=pv.rearrange("p (h d) -> p h d", h=HPC),
                in1=bv_b.rearrange("p (h d) -> p h d", h=HPC),
            )

        def transpose_tile(st):
            for half in range(2):
                tp = tp_p.tile([128, 4, 128], f32, tag="tp",
                               name=f"tp{st}_{half}")
                for k4 in range(4):
                    k = half * 4 + k4
                    nc.tensor.transpose(
                        out=tp[:, k4, :],
                        in_=x_tiles[st][:, k * 128:(k + 1) * 128],
                        identity=ident,
                    )
                nc.vector.tensor_copy(
                    out=xt_r[:, half * 4:half * 4 + 4, st * 128:(st + 1) * 128],
                    in_=tp,
                )

        # ---- attention + output projection ----
        with ExitStack() as ph2:
            # ---- minimal pre-work so attention (0,0) can start ----
            for st in range(4):
                transpose_tile(st)
            proj_qk(ps_x, wk_r, bk_sb, kt_r, 0, 0)
            proj_qk(ps_x, wq_r, bq_sb, qt_r, 0, 0)
            nc.vector.tensor_copy(out=wo_r, in_=wstg["wo"])

            # Filler queue: exp-independent PE work (out-projections, next-j Q
            # projections) dripped into the attention t-loop so the PE never
            # idles waiting on ACT and the HAM clock stays warm.
            filler = []

            def feed(n):
                for _ in range(min(n, len(filler))):
                    filler.pop(0)()

            def flush():
                while filler:
                    filler.pop(0)()

            def queue_proj_qk(wr, bias_sb, dst, c, j):
                state = {}
                def mk(k):
                    def go():
                        if k == 0:
                            state["pq"] = ps_x.tile(
                                [128, SBK], f32, tag="px", name=f"fq{c}_{j}_{k}")
                        nc.tensor.matmul(
                            out=state["pq"],
                            lhsT=wr[:, k, c * 128:(c + 1) * 128],
                            rhs=xt_r[:, k, j * SBK:(j + 1) * SBK],
                            start=(k == 0), stop=(k == NDC - 1),
                        )
                        if k == NDC - 1:
                            nc.vector.tensor_scalar_add(
                                out=dst[:, c, j * SBK:(j + 1) * SBK],
                                in0=state["pq"], scalar1=bias_sb[:, c:c + 1],
                            )
                    return go
                for k in range(NDC):
                    filler.append(mk(k))

            def queue_outproj(j):
                for stj in range(SBK // 128):
                    st = j * (SBK // 128) + stj
                    for nh in range(2):
                        state = {}
                        def mk(c, st=st, nh=nh, state=state):
                            def go():
                                if c == 0:
                                    state["po"] = ps_x.tile(
                                        [128, 512], f32, tag="px", name=f"po{st}_{nh}")
                                nc.tensor.matmul(
                                    out=state["po"],
                                    lhsT=ct_r[:, c, st * 128:(st + 1) * 128],
                                    rhs=wo_r[:, c, nh * 512:(nh + 1) * 512],
                                    start=(c == 0), stop=(c == NPC - 1),
                                )
                                if c == NPC - 1:
                                    ob = outp.tile([128, 512], f32, tag="ob",
                                                   name=f"ob{st}_{nh}")
                                    nc.vector.tensor_copy(out=ob, in_=state["po"])
                                    eng = nc.sync if st % 2 == 0 else nc.scalar
                                    eng.dma_start(
                                        out=out[st * 128:(st + 1) * 128,
                                                nh * 512:(nh + 1) * 512],
                                        in_=ob)
                            return go
                        for c in range(NPC):
                            filler.append(mk(c))

            def normalize(j, hp, pcs):
                # pcs[hh] rows 64-127 hold the softmax denominator replicated
                # per partition (ones-block matmul output): approx-reciprocal
                # (~5x faster than reciprocal(), 18-bit accurate) + mul, both
                # DVE, reading straight from PSUM. (Full reciprocal() is ~12
                # cyc/elem and stalled the PE at every phase boundary; DVE
                # divide doesn't compile on HW.)
                for hh in range(2):
                    dn = smal.tile([64, SBK], f32, tag=f"dn{hh}",
                                   name=f"dn{j}{hp}{hh}")
                    nc.vector.tensor_copy(out=dn, in_=pcs[hh][64:128, :])
                    rc = smal.tile([64, SBK], f32, tag=f"rc{hh}",
                                   name=f"rc{j}{hp}{hh}")
                    nc.vector.reciprocal_approx_fast(out=rc, in_=dn)
                    nc.vector.tensor_mul(
                        out=ct_r[hh * 64:(hh + 1) * 64, hp, j * SBK:(j + 1) * SBK],
                        in0=pcs[hh][0:64, :],
                        in1=rc,
                    )

            # Software-pipelined driver (depth 2): at step i emit S(i), exp(i),
            # then the PV of step i-2 — whose exp finished two steps ago, so
            # the PE never sits in an exp-wait ahead of the next S and the exp
            # stream stays back-to-back.
            pend = []

            def drain_pv():
                if not pend:
                    return
                j, hp, t, es, pcs = pend.pop(0)
                for hh in range(2):
                    nc.tensor.matmul(
                        out=pcs[hh],
                        lhsT=vaug[:, t, hp * 2 + hh, :],
                        rhs=es[:, hh, :],
                        start=(t == 0), stop=(t == NT - 1),
                    )
                if t == NT - 1:
                    normalize(j, hp, pcs)
                    if hp == 1:
                        queue_outproj(j)

            pcs_by = {}
            for j in range(NSB):
                for hp in range(NPC):
                    first = (j, hp) == (0, 0)
                    if (j, hp) == (0, 1):
                        # ensure the c=1 K projections (queued at (0,0)) are
                        # all emitted before this phase's first scores matmul,
                        # then retire the fold pools and bring up the
                        # double-buffered scores PSUM.
                        flush()
                        fold.close()
                        pools["ss"] = top.enter_context(
                            tc.tile_pool(name="ps_s", bufs=2, space="PSUM"))
                    pcs_by[(j, hp)] = [
                        ps_c.tile([128, SBK], f32, tag=f"pc{hh}", name=f"pc{hh}_{j}_{hp}")
                        for hh in range(2)]
                    if first:
                        # c=1 K blocks for j>=1 are queued inside the t-loop,
                        # after their x tiles' transposes are emitted.
                        queue_proj_qk(wq_r, bq_sb, qt_r, 1, 0)
                        queue_proj_qk(wk_r, bk_sb, kt_r, 1, 0)
                    if hp == 1 and j + 1 < NSB:
                        for c in range(NPC):
                            queue_proj_qk(wq_r, bq_sb, qt_r, c, j + 1)
                    for t in range(NT):
                        if first:
                            if t < 12:
                                transpose_tile(t + 4)
                                if t % 4 == 3:
                                    proj_qk(ps_x, wk_r, bk_sb, kt_r, 0, t // 4 + 1)
                                    queue_proj_qk(wk_r, bk_sb, kt_r, 1, t // 4 + 1)
                            proj_v(ps_x, t)        # V projection rides along
                        pool_ss = ss0_p if first else pools["ss"]
                        ss = pool_ss.tile([128, 2, SBK], f32, tag="ss", name=f"ss{j}_{hp}_{t}")
                        for hh in range(2):
                            nc.tensor.matmul(
                                out=ss[:, hh, :],
                                lhsT=kt_r[hh * 64:(hh + 1) * 64, hp, t * 128:(t + 1) * 128],
                                rhs=qt_r[hh * 64:(hh + 1) * 64, hp, j * SBK:(j + 1) * SBK],
                                start=True, stop=True,
                            )
                        es = esp.tile([128, 2, SBK], bf16, tag="es", name=f"es{j}_{hp}_{t}")
                        nc.scalar.activation(out=es, in_=ss, func=AF.Exp, scale=0.125)
                        if len(pend) >= 2:
                            drain_pv()
                        pend.append((j, hp, t, es, pcs_by[(j, hp)]))
                        feed(3)
            drain_pv()
            drain_pv()
            flush()

    nc.finalize()
    return nc


def _get_program():
    if "nc" not in _prog_cache:
        _prog_cache["nc"] = _build_program()
    return _prog_cache["nc"]


def _make_in_maps(x, Wq, bq, Wk, bk, Wv, bv, Wo, bo):
    in_maps = []
    for c in range(NCORES):
        b, hg = divmod(c, TP)
        sl = slice(hg * DP, (hg + 1) * DP)
        in_maps.append({
            "x": np.ascontiguousarray(x[b]),
            "wq": np.ascontiguousarray(Wq[:, sl]),
            "wk": np.ascontiguousarray(Wk[:, sl]),
            "wv": np.ascontiguousarray(Wv[:, sl]),
            "wo": np.ascontiguousarray(Wo[sl, :]),
            "bq": np.ascontiguousarray(bq[sl]),
            "bk": np.ascontiguousarray(bk[sl]),
            "bv": np.ascontiguousarray(bv[sl]),
        })
    return in_maps


def run(inputs, **spmd_kwargs):
    """Build, run on 8 cores, gather. Returns (output, BassKernelResults)."""
    args = {k: np.asarray(v, dtype=np.float32) for k, v in inputs.items()}
    nc = _get_program()
    in_maps = _make_in_maps(
        args["x"], args["Wq"], args["bq"], args["Wk"], args["bk"],
        args["Wv"], args["bv"], args["Wo"], args["bo"],
    )
    res = run_bass_kernel_spmd(nc, in_maps, list(range(NCORES)), **spmd_kwargs)
    out = np.zeros((B, S, D), dtype=np.float32)
    for c in range(NCORES):
        b = c // TP
        out[b] += res.results[c]["out"]
    out += args["bo"]
    return out, res


def kernel(**inputs):
    out, _ = run(inputs)
    return out



# revision 31
# speedup vs baseline: 1.4772x; 1.0227x over previous
"""Multi-head attention (B=2, S=2048, D=1024, H=16, Dk=64) on 8 NeuronCores.

Sharding: 2-way data parallel over batch x 4-way tensor parallel over heads.
Core c handles batch c//4 and heads (c%4)*4 .. (c%4)*4+3, i.e. a 256-column
slice of the QKV projections and the matching 256-row slice of Wo. Each core
computes a partial output projection [S, D]; the host sums the 4 partials per
batch (the all-reduce of the sharding hint) and stacks the batches.

On-core algorithm (matmuls in float32r = full-rate fp32, PSUM accum fp32):
  x^T via PE transpose -> Q^T, K^T head-packed [128, 2, S] (head parity on
  partition halves 0-63/64-127 so the two heads' K=64 score matmuls run
  concurrently in separate PE row groups) and V in natural [t, d'] layout,
  augmented with a ones column -> S^T = K_h Q_h^T -> exp on ACT (1/8 scale
  folded into the activation; no max subtraction needed: scores are O(5)
  for unit-variance inputs, far from fp32 overflow) -> C^T = V_aug^T @
  expS^T where the ones row yields the softmax denominator for free ->
  normalize -> partial out = C^T.T @ Wo_slice + bo/4.

Emission order interleaves the V projection and the j+1 Q projection into
attention block j so the PE keeps dense work while ACT grinds through the
exp stream; x/output DMAs are split across the SP/ACT HWDGE queues and
weight loads go via the gpsimd SWDGE queues.
"""
from contextlib import ExitStack

import numpy as np
import concourse.bass as bass
import concourse.mybir as mybir
import concourse.tile as tile
from concourse import bacc
from concourse.bass_utils import run_bass_kernel_spmd

f32 = mybir.dt.float32
f32r = mybir.dt.float32r
bf16 = mybir.dt.bfloat16
AF = mybir.ActivationFunctionType
ALU = mybir.AluOpType

B, S, D = 2, 2048, 1024
H, DK = 16, 64
NCORES = 8
TP = 4                 # tensor-parallel factor (head groups)
HPC = H // TP          # 4 heads per core
DP = HPC * DK          # 256 = per-core d' slice
SBK = 512              # s-block for attention streaming
NSB = S // SBK         # 4
NT = S // 128          # 16 t-tiles
NDC = D // 128         # 8 contraction chunks over D
NPC = DP // 128        # 2 chunks over d'

_prog_cache = {}


def _build_program():
    nc = bacc.Bacc()
    # x and the identity are f32r so the PE transposes run at 1.5 cyc/row
    # (f32 transposes are 2 cyc/row); bit-identical storage to f32.
    x = nc.dram_tensor("x", [S, D], f32r, kind="ExternalInput")
    wq = nc.dram_tensor("wq", [D, DP], f32, kind="ExternalInput")
    wk = nc.dram_tensor("wk", [D, DP], f32, kind="ExternalInput")
    wv = nc.dram_tensor("wv", [D, DP], f32, kind="ExternalInput")
    wo = nc.dram_tensor("wo", [DP, D], f32, kind="ExternalInput")
    # host-prepared: identity for PE transposes, partition-major biases, and
    # the V bias pre-broadcast across partitions (avoids the gpsimd engine
    # entirely -- it has a ~6.5us cold-start that gated the old startup).
    identm = nc.dram_tensor("identm", [128, 128], f32r, kind="ExternalInput")
    bqs = nc.dram_tensor("bqs", [128, NPC], f32, kind="ExternalInput")
    bks = nc.dram_tensor("bks", [128, NPC], f32, kind="ExternalInput")
    bvb = nc.dram_tensor("bvb", [128, DP], f32, kind="ExternalInput")
    out = nc.dram_tensor("out", [S, D], f32, kind="ExternalOutput")

    with tile.TileContext(nc) as tc, ExitStack() as top:
        const = top.enter_context(tc.tile_pool(name="const", bufs=1))
        big = top.enter_context(tc.tile_pool(name="big", bufs=1))
        xtp = top.enter_context(tc.tile_pool(name="xt", bufs=1))

        ident = const.tile([128, 128], f32r)

        # persistent activations
        qt_r = big.tile([128, NPC, S], bf16)
        kt_r = big.tile([128, NPC, S], bf16)
        # V augmented with a 64-wide ones block: the PV matmul then emits the
        # softmax denominator replicated across PSUM partitions 64-127, so
        # normalization is a straight DVE reciprocal+mul from PSUM (no gpsimd
        # transpose/broadcast round-trips).
        vaug = big.tile([128, NT, HPC, 2 * DK], bf16)
        ct_r = big.tile([128, NPC, S], bf16)
        xt_r = xtp.tile([128, NDC, S], bf16)

        wq_r = const.tile([128, NDC, DP], bf16)
        wv_r = const.tile([128, NDC, DP], bf16)
        wo_r = const.tile([128, NPC, D], bf16)
        bq_sb = const.tile([128, NPC], f32)
        bk_sb = const.tile([128, NPC], f32)
        bv_b = const.tile([128, DP], f32)

        wkp = top.enter_context(tc.tile_pool(name="wkp", bufs=1))
        wk_r = wkp.tile([128, NDC, DP], bf16)

        # attention-phase pools (created before the fold pools so the fold
        # stack can be released mid-loop in LIFO order)
        esp = top.enter_context(tc.tile_pool(name="esp", bufs=3))
        smal = top.enter_context(tc.tile_pool(name="smal", bufs=2))
        outp = top.enter_context(tc.tile_pool(name="outp", bufs=2))
        ps_x = top.enter_context(tc.tile_pool(name="ps_x", bufs=2, space="PSUM"))
        ps_c = top.enter_context(tc.tile_pool(name="ps_c", bufs=1, space="PSUM"))
        pools = {}   # ps_s (bufs=2) is created once the fold pools close

        # Fold-phase pools: startup (x transposes, c=0 K/Q projections) is
        # folded into the first attention phase (0,0) so the exp stream and
        # the PE pipeline start ~70us earlier. These close at phase (0,1).
        fold = ExitStack()
        stg = fold.enter_context(tc.tile_pool(name="stg", bufs=2))
        xin = fold.enter_context(tc.tile_pool(name="xin", bufs=2))
        tp_p = fold.enter_context(tc.tile_pool(name="tp", bufs=2, space="PSUM"))
        ss0_p = fold.enter_context(tc.tile_pool(name="ss0", bufs=1, space="PSUM"))

        # ---- loads on the two HWDGE rings (SP + ACT), interleaved with the
        # x tiles in first-use order: identity first (gates every transpose),
        # then biases, then weights as their consumers come up ----
        wstg = {}

        def stage_w(src, npc, nm, eng):
            sf = stg.tile([128, npc, src.shape[1]], f32, tag="wstg",
                          name=f"stg_{nm}")
            eng.dma_start(out=sf,
                          in_=src.rearrange("(ko ki) d -> ki ko d", ki=128))
            wstg[nm] = sf

        x_tiles = [xin.tile([128, D], f32r, tag="x_t", name=f"x_t{st}")
                   for st in range(NT)]

        def load_x(st):
            eng = nc.sync if st % 2 == 0 else nc.scalar
            eng.dma_start(out=x_tiles[st], in_=x[st * 128:(st + 1) * 128, :])

        nc.sync.dma_start(out=ident, in_=identm[:, :])
        nc.scalar.dma_start(out=bq_sb, in_=bqs[:, :])
        load_x(0); load_x(1); load_x(2); load_x(3)
        nc.sync.dma_start(out=bk_sb, in_=bks[:, :])
        nc.scalar.dma_start(out=bv_b, in_=bvb[:, :])
        stage_w(wk, NDC, "wk", nc.sync)
        stage_w(wq, NDC, "wq", nc.scalar)
        load_x(4); load_x(5)
        stage_w(wv, NDC, "wv", nc.sync)
        load_x(6); load_x(7)
        stage_w(wo, NPC, "wo", nc.scalar)
        for st in range(8, NT):
            load_x(st)

        nc.vector.tensor_copy(out=wk_r, in_=wstg["wk"])
        nc.vector.tensor_copy(out=wq_r, in_=wstg["wq"])
        nc.vector.tensor_copy(out=wv_r, in_=wstg["wv"])

        def proj_qk(pool, wr, bias_sb, dst, c, j):
            pq = pool.tile([128, SBK], f32, tag="px", name=f"pj{c}_{j}_{id(wr)%89}")
            for k in range(NDC):
                nc.tensor.matmul(
                    out=pq,
                    lhsT=wr[:, k, c * 128:(c + 1) * 128],
                    rhs=xt_r[:, k, j * SBK:(j + 1) * SBK],
                    start=(k == 0), stop=(k == NDC - 1),
                )
            nc.vector.tensor_scalar_add(
                out=dst[:, c, j * SBK:(j + 1) * SBK],
                in0=pq, scalar1=bias_sb[:, c:c + 1],
            )

        def proj_v(pool, st):
            pv = pool.tile([128, DP], f32, tag="px", name=f"pv{st}")
            for k in range(NDC):
                nc.tensor.matmul(
                    out=pv,
                    lhsT=xt_r[:, k, st * 128:(st + 1) * 128],
                    rhs=wv_r[:, k, :],
                    start=(k == 0), stop=(k == NDC - 1),
                )
            nc.vector.tensor_add(
                out=vaug[:, st, :, 0:DK],
                in0=pv.rearrange("p (h d) -> p h d", h=HPC),
                in1=bv_b.rearrange("p (h d) -> p h d", h=HPC),
            )
            nc.vector.memset(vaug[:, st, :, DK:2 * DK], 1.0)

        def transpose_tile(st):
            for half in range(2):
                tp = tp_p.tile([128, 4, 128], f32r, tag="tp",
                               name=f"tp{st}_{half}")
                for k4 in range(4):
                    k = half * 4 + k4
                    nc.tensor.transpose(
                        out=tp[:, k4, :],
                        in_=x_tiles[st][:, k * 128:(k + 1) * 128],
                        identity=ident,
                    )
                nc.vector.tensor_copy(
                    out=xt_r[:, half * 4:half * 4 + 4, st * 128:(st + 1) * 128],
                    in_=tp,
                )

        # ---- attention + output projection ----
        with ExitStack() as ph2:
            # ---- minimal pre-work so attention (0,0) can start ----
            for st in range(4):
                transpose_tile(st)
            proj_qk(ps_x, wk_r, bk_sb, kt_r, 0, 0)
            proj_qk(ps_x, wq_r, bq_sb, qt_r, 0, 0)
            nc.vector.tensor_copy(out=wo_r, in_=wstg["wo"])

            # Filler queue: exp-independent PE work (out-projections, next-j Q
            # projections) dripped into the attention t-loop so the PE never
            # idles waiting on ACT and the HAM clock stays warm.
            filler = []

            def feed(n):
                for _ in range(min(n, len(filler))):
                    filler.pop(0)()

            def flush():
                while filler:
                    filler.pop(0)()

            def queue_proj_qk(wr, bias_sb, dst, c, j):
                state = {}
                def mk(k):
                    def go():
                        if k == 0:
                            state["pq"] = ps_x.tile(
                                [128, SBK], f32, tag="px", name=f"fq{c}_{j}_{k}")
                        nc.tensor.matmul(
                            out=state["pq"],
                            lhsT=wr[:, k, c * 128:(c + 1) * 128],
                            rhs=xt_r[:, k, j * SBK:(j + 1) * SBK],
                            start=(k == 0), stop=(k == NDC - 1),
                        )
                        if k == NDC - 1:
                            nc.vector.tensor_scalar_add(
                                out=dst[:, c, j * SBK:(j + 1) * SBK],
                                in0=state["pq"], scalar1=bias_sb[:, c:c + 1],
                            )
                    return go
                for k in range(NDC):
                    filler.append(mk(k))

            def queue_outproj(j):
                for stj in range(SBK // 128):
                    st = j * (SBK // 128) + stj
                    for nh in range(2):
                        state = {}
                        def mk(c, st=st, nh=nh, state=state):
                            def go():
                                if c == 0:
                                    state["po"] = ps_x.tile(
                                        [128, 512], f32, tag="px", name=f"po{st}_{nh}")
                                nc.tensor.matmul(
                                    out=state["po"],
                                    lhsT=ct_r[:, c, st * 128:(st + 1) * 128],
                                    rhs=wo_r[:, c, nh * 512:(nh + 1) * 512],
                                    start=(c == 0), stop=(c == NPC - 1),
                                )
                                if c == NPC - 1:
                                    ob = outp.tile([128, 512], f32, tag="ob",
                                                   name=f"ob{st}_{nh}")
                                    nc.vector.tensor_copy(out=ob, in_=state["po"])
                                    eng = nc.sync if st % 2 == 0 else nc.scalar
                                    eng.dma_start(
                                        out=out[st * 128:(st + 1) * 128,
                                                nh * 512:(nh + 1) * 512],
                                        in_=ob)
                            return go
                        for c in range(NPC):
                            filler.append(mk(c))

            def normalize(j, hp, pcs):
                # pcs[hh] rows 64-127 hold the softmax denominator replicated
                # per partition (ones-block matmul output): approx-reciprocal
                # (~5x faster than reciprocal(), 18-bit accurate) + mul, both
                # DVE, reading straight from PSUM. (Full reciprocal() is ~12
                # cyc/elem and stalled the PE at every phase boundary; DVE
                # divide doesn't compile on HW.)
                for hh in range(2):
                    dn = smal.tile([64, SBK], f32, tag=f"dn{hh}",
                                   name=f"dn{j}{hp}{hh}")
                    nc.vector.tensor_copy(out=dn, in_=pcs[hh][64:128, :])
                    rc = smal.tile([64, SBK], f32, tag=f"rc{hh}",
                                   name=f"rc{j}{hp}{hh}")
                    nc.vector.reciprocal_approx_fast(out=rc, in_=dn)
                    nc.vector.tensor_mul(
                        out=ct_r[hh * 64:(hh + 1) * 64, hp, j * SBK:(j + 1) * SBK],
                        in0=pcs[hh][0:64, :],
                        in1=rc,
                    )

            # Software-pipelined driver (depth 2): at step i emit S(i), exp(i),
            # then the PV of step i-2 — whose exp finished two steps ago, so
            # the PE never sits in an exp-wait ahead of the next S and the exp
            # stream stays back-to-back.
            pend = []

            def tail_norm_outproj(j, hp, pcs):
                # Final phase: chunk the normalize by 128-column groups and
                # emit each output-projection tile right behind its chunk, so
                # the PE never sits through a full-width DVE chain at the end.
                for c4 in range(SBK // 128):
                    st = j * (SBK // 128) + c4
                    cs = slice(c4 * 128, (c4 + 1) * 128)
                    for hh in range(2):
                        dn = smal.tile([64, 128], f32, tag=f"tdn{hh}",
                                       name=f"tdn{c4}_{hh}")
                        nc.vector.tensor_copy(out=dn, in_=pcs[hh][64:128, cs])
                        rc = smal.tile([64, 128], f32, tag=f"trc{hh}",
                                       name=f"trc{c4}_{hh}")
                        nc.vector.reciprocal_approx_fast(out=rc, in_=dn)
                        nc.vector.tensor_mul(
                            out=ct_r[hh * 64:(hh + 1) * 64, hp,
                                     j * SBK + c4 * 128:j * SBK + (c4 + 1) * 128],
                            in0=pcs[hh][0:64, cs],
                            in1=rc,
                        )
                    for nh in range(2):
                        po = ps_x.tile([128, 512], f32, tag="px",
                                       name=f"tpo{st}_{nh}")
                        for c in range(NPC):
                            nc.tensor.matmul(
                                out=po,
                                lhsT=ct_r[:, c, st * 128:(st + 1) * 128],
                                rhs=wo_r[:, c, nh * 512:(nh + 1) * 512],
                                start=(c == 0), stop=(c == NPC - 1),
                            )
                        ob = outp.tile([128, 512], f32, tag="ob",
                                       name=f"tob{st}_{nh}")
                        nc.vector.tensor_copy(out=ob, in_=po)
                        eng = nc.sync if st % 2 == 0 else nc.scalar
                        eng.dma_start(
                            out=out[st * 128:(st + 1) * 128,
                                    nh * 512:(nh + 1) * 512],
                            in_=ob)

            def drain_pv():
                if not pend:
                    return
                j, hp, t, es, pcs = pend.pop(0)
                for hh in range(2):
                    nc.tensor.matmul(
                        out=pcs[hh],
                        lhsT=vaug[:, t, hp * 2 + hh, :],
                        rhs=es[:, hh, :],
                        start=(t == 0), stop=(t == NT - 1),
                    )
                if t == NT - 1:
                    if (j, hp) == (NSB - 1, 1):
                        flush()
                        tail_norm_outproj(j, hp, pcs)
                    else:
                        normalize(j, hp, pcs)
                        if hp == 1:
                            queue_outproj(j)

            pcs_by = {}
            for j in range(NSB):
                for hp in range(NPC):
                    first = (j, hp) == (0, 0)
                    if (j, hp) == (0, 1):
                        # ensure the c=1 K projections (queued at (0,0)) are
                        # all emitted before this phase's first scores matmul,
                        # then retire the fold pools and bring up the
                        # double-buffered scores PSUM.
                        flush()
                        fold.close()
                        pools["ss"] = top.enter_context(
                            tc.tile_pool(name="ps_s", bufs=2, space="PSUM"))
                    pcs_by[(j, hp)] = [
                        ps_c.tile([128, SBK], f32, tag=f"pc{hh}", name=f"pc{hh}_{j}_{hp}")
                        for hh in range(2)]
                    if first:
                        # c=1 K blocks for j>=1 are queued inside the t-loop,
                        # after their x tiles' transposes are emitted.
                        queue_proj_qk(wq_r, bq_sb, qt_r, 1, 0)
                        queue_proj_qk(wk_r, bk_sb, kt_r, 1, 0)
                    if hp == 1 and j + 1 < NSB:
                        for c in range(NPC):
                            queue_proj_qk(wq_r, bq_sb, qt_r, c, j + 1)
                    for t in range(NT):
                        if first:
                            if t < 12:
                                transpose_tile(t + 4)
                                if t % 4 == 3:
                                    proj_qk(ps_x, wk_r, bk_sb, kt_r, 0, t // 4 + 1)
                                    queue_proj_qk(wk_r, bk_sb, kt_r, 1, t // 4 + 1)
                            proj_v(ps_x, t)        # V projection rides along
                        pool_ss = ss0_p if first else pools["ss"]
                        ss = pool_ss.tile([128, 2, SBK], f32, tag="ss", name=f"ss{j}_{hp}_{t}")
                        for hh in range(2):
                            nc.tensor.matmul(
                                out=ss[:, hh, :],
                                lhsT=kt_r[hh * 64:(hh + 1) * 64, hp, t * 128:(t + 1) * 128],
                                rhs=qt_r[hh * 64:(hh + 1) * 64, hp, j * SBK:(j + 1) * SBK],
                                start=True, stop=True,
                            )
                        es = esp.tile([128, 2, SBK], bf16, tag="es", name=f"es{j}_{hp}_{t}")
                        nc.scalar.activation(out=es, in_=ss, func=AF.Exp, scale=0.125)
                        if len(pend) >= 2:
                            drain_pv()
                        pend.append((j, hp, t, es, pcs_by[(j, hp)]))
                        feed(3)
            drain_pv()
            drain_pv()
            flush()

    nc.finalize()
    return nc


def _get_program():
    if "nc" not in _prog_cache:
        _prog_cache["nc"] = _build_program()
    return _prog_cache["nc"]


def _make_in_maps(x, Wq, bq, Wk, bk, Wv, bv, Wo, bo):
    ident = np.eye(128, dtype=np.float32)
    in_maps = []
    for c in range(NCORES):
        b, hg = divmod(c, TP)
        sl = slice(hg * DP, (hg + 1) * DP)
        in_maps.append({
            "x": np.ascontiguousarray(x[b]),
            "wq": np.ascontiguousarray(Wq[:, sl]),
            "wk": np.ascontiguousarray(Wk[:, sl]),
            "wv": np.ascontiguousarray(Wv[:, sl]),
            "wo": np.ascontiguousarray(Wo[sl, :]),
            "identm": ident,
            "bqs": np.ascontiguousarray(bq[sl].reshape(NPC, 128).T),
            "bks": np.ascontiguousarray(bk[sl].reshape(NPC, 128).T),
            "bvb": np.ascontiguousarray(
                np.broadcast_to(bv[sl], (128, DP))),
        })
    return in_maps


def run(inputs, **spmd_kwargs):
    """Build, run on 8 cores, gather. Returns (output, BassKernelResults)."""
    args = {k: np.asarray(v, dtype=np.float32) for k, v in inputs.items()}
    nc = _get_program()
    in_maps = _make_in_maps(
        args["x"], args["Wq"], args["bq"], args["Wk"], args["bk"],
        args["Wv"], args["bv"], args["Wo"], args["bo"],
    )
    res = run_bass_kernel_spmd(nc, in_maps, list(range(NCORES)), **spmd_kwargs)
    out = np.zeros((B, S, D), dtype=np.float32)
    for c in range(NCORES):
        b = c // TP
        out[b] += res.results[c]["out"]
    out += args["bo"]
    return out, res


def kernel(**inputs):
    out, _ = run(inputs)
    return out

